# revision 1
# baseline (speedup 1.0000x reference)
"""Trainium2 Bass kernel for nn_CSI_75453985457421 (LN + chunked Mamba + MLP + 1x1conv + BN + SiLU).

Sharding: 8 cores = (batch b 0..3) x (time-half 0..1). Each core gets
x[b, :, half*2048-67 : half*2048+2048] (zero-padded before the sequence start)
and computes its 2048 output positions independently: 67 warmup columns
(3 causal-conv pad + 64 scan warmup; state decay <= exp(-0.68*64) << fp32 eps).

Device layout: time on the free axis. The selective scan runs with partitions
= (d_local, s): 16 groups of 8 d-channels x 16 states via the hardware
tensor_tensor_scan (DVE). dt/dtu/B/C broadcasts and the final sum over s are
TensorE pattern matmuls; exp(A*dt) is ScalarE with a per-partition scale.
LN gamma/beta, the depthwise conv, the channel interleave and BatchNorm are
folded into weights on the host.
"""
import os
import sys

sys.path.insert(0, "/opt/trn_rl_repo")
STAGE = int(os.environ.get("KSTAGE", "9"))
import numpy as np
import concourse.bass as bass
import concourse.bacc as bacc
import concourse.tile as tile
from concourse import mybir
from concourse.bass_utils import run_bass_kernel_spmd

F32 = mybir.dt.float32
AOT = mybir.AluOpType
AFT = mybir.ActivationFunctionType

B, C, H, W = 4, 256, 64, 64
N = H * W
D, DI, DS, DC, DTR, MH = 64, 128, 16, 4, 4, 256
EPS = 1e-5
PAD = 67
TH = 2048
TEXT = PAD + TH          # 2115
SCT = TEXT - 3           # 2112 = 4*528
SUB = 528
OSUB = 512

_cache = {}

_IN_SHAPES = dict(
    xs=(C, TEXT), wctap=(128, 16 * DI), wz=(128, 4 * DI), ccv=(DI, 4), cz=(DI, 4),
    xpw=(DI, 96), dtw=(DTR, DI), dtb=(DI, 1), acols=(128, 16), dp=(DI, 1),
    opw=(DI, D), fc1=(D, MH), fc1b=(128, 2), fc2=(128, 2 * D), fc2b=(128, 1),
    wout=(128, 2 * C), bnsc=(128, 2), bnsh=(128, 2), patg=(128, 16 * 128),
    patyg=(128, 16 * 128), patsbc=(128, 256), ones1=(1, 128), onesc=(128, 1),
    skips=(128, 1),
)


def _build():
    if "nc" in _cache:
        return _cache["nc"]
    nc = bacc.Bacc("TRN2", target_bir_lowering=False, debug=False, num_devices=8)
    dram = {k: nc.dram_tensor(k, list(s), F32, kind="ExternalInput").ap()
            for k, s in _IN_SHAPES.items()}
    out = nc.dram_tensor("out", [C, TH], F32, kind="ExternalOutput").ap()

    with tile.TileContext(nc) as tc, \
            tc.tile_pool(name="const", bufs=1) as Kp, \
            tc.tile_pool(name="big", bufs=1) as Bp, \
            tc.tile_pool(name="seq", bufs=1) as Sp, \
            tc.tile_pool(name="tmp", bufs=2) as Tp, \
            tc.tile_pool(name="scan", bufs=3) as Cp, \
            tc.tile_pool(name="psA", bufs=1, space="PSUM") as psA, \
            tc.tile_pool(name="psM", bufs=1, space="PSUM") as psM, \
            tc.tile_pool(name="psY", bufs=1, space="PSUM") as psY:

        def mm(out_ap, lhsT, rhs, start=True, stop=True):
            n = out_ap.shape[-1]
            if n <= 512:
                nc.tensor.matmul(out_ap, lhsT, rhs, start=start, stop=stop)
                return
            o = 0
            while o < n:
                w_ = min(512, n - o)
                nc.tensor.matmul(out_ap[..., o:o + w_], lhsT, rhs[..., o:o + w_],
                                 start=start, stop=stop)
                o += w_

        ct = {}
        for k in _IN_SHAPES:
            if k == "xs":
                continue
            ct[k] = Kp.tile(list(_IN_SHAPES[k]), F32, tag=k, name=f"ct_{k}")
            nc.sync.dma_start(out=ct[k][:], in_=dram[k][:])
        eps_t = Kp.tile([1, 1], F32, tag="eps")
        nc.vector.memset(eps_t[:], EPS)

        xh = [Bp.tile([128, TEXT], F32, tag=f"xh{h}", name=f"xh{h}") for h in range(2)]
        for h in range(2):
            nc.sync.dma_start(out=xh[h][:], in_=dram["xs"][128 * h:128 * (h + 1), :])

        # ---- LayerNorm over C: fused per-subtile stats + apply ----
        nsub = [(i * 512, min(512, TEXT - i * 512)) for i in range((TEXT + 511) // 512)]
        for (o, w_) in nsub:
            pse = psM.tile([1, 512], F32, tag="pmm")
            for h in range(2):
                mm(pse[:, :w_], ct["onesc"][:], xh[h][:, o:o + w_],
                   start=(h == 0), stop=(h == 1))
            mean = Tp.tile([1, 512], F32, tag="rA")
            nc.vector.tensor_scalar(out=mean[:, :w_], in0=pse[:, :w_],
                                    scalar1=1.0 / C, scalar2=None, op0=AOT.mult)
            psq = psM.tile([1, 512], F32, tag="pmm")
            for h in range(2):
                sqt = Tp.tile([128, 512], F32, tag="scr")
                nc.scalar.activation(sqt[:, :w_], xh[h][:, o:o + w_], AFT.Square)
                mm(psq[:, :w_], ct["onesc"][:], sqt[:, :w_],
                   start=(h == 0), stop=(h == 1))
            sqm = Tp.tile([1, 512], F32, tag="rB")
            nc.vector.tensor_scalar(out=sqm[:, :w_], in0=psq[:, :w_],
                                    scalar1=1.0 / C, scalar2=None, op0=AOT.mult)
            m2 = Tp.tile([1, 512], F32, tag="rC")
            nc.vector.tensor_tensor(m2[:, :w_], mean[:, :w_], mean[:, :w_], AOT.mult)
            var = Tp.tile([1, 512], F32, tag="rD")
            nc.vector.tensor_tensor(var[:, :w_], sqm[:, :w_], m2[:, :w_], AOT.subtract)
            sd = Tp.tile([1, 512], F32, tag="rC")
            nc.scalar.activation(sd[:, :w_], var[:, :w_], AFT.Sqrt, bias=eps_t[:])
            rstd = Tp.tile([1, 512], F32, tag="rD")
            nc.vector.reciprocal_approx_fast(rstd[:, :w_], sd[:, :w_])
            pmb = psA.tile([128, SUB], F32, tag="pbc")
            mm(pmb[:, :w_], ct["ones1"][:], mean[:, :w_])
            prb = psM.tile([128, SUB], F32, tag="pmm")
            mm(prb[:, :w_], ct["ones1"][:], rstd[:, :w_])
            for h in range(2):
                tmp = Tp.tile([128, 512], F32, tag="scr")
                nc.vector.scalar_tensor_tensor(tmp[:, :w_], xh[h][:, o:o + w_], 1.0,
                                               pmb[:, :w_], AOT.mult, AOT.subtract)
                nc.vector.scalar_tensor_tensor(xh[h][:, o:o + w_], tmp[:, :w_], 1.0,
                                               prb[:, :w_], AOT.mult, AOT.mult)

        mfin = [Bp.tile([128, TH], F32, tag=f"mfin{h}", name=f"mfin{h}") for h in range(2)]
        if STAGE <= 1:
            for half in range(2):
                nc.sync.dma_start(out=out[128 * half:128 * (half + 1), :],
                                  in_=xh[half][:, PAD:])
        nseq = 0 if STAGE <= 1 else 4
        # ==== per sequence (channel chunk) ====
        for i in range(nseq):
            xnh = xh[i // 2]
            r0 = 64 * (i % 2)
            xcT = Sp.tile([128, SCT], F32, tag="xcT")
            szT = Sp.tile([128, SCT], F32, tag="szT")
            dtT = Sp.tile([128, SCT], F32, tag="dtT")
            dtuT = Sp.tile([128, SCT], F32, tag="dtuT")
            BbT = Sp.tile([128, SCT], F32, tag="BbT")
            CbT = Sp.tile([128, SCT], F32, tag="CbT")

            for c in range(4):
                o = SUB * c
                pxt = psA.tile([128, SUB], F32, tag="pbc")
                for j in range(DC):
                    mm(pxt[:], ct["wctap"][r0:r0 + 64, (4 * i + j) * DI:(4 * i + j + 1) * DI],
                       xnh[r0:r0 + 64, o + j:o + j + SUB],
                       start=(j == 0), stop=(j == DC - 1))
                nc.scalar.activation(xcT[:, o:o + SUB], pxt[:], AFT.Silu,
                                     bias=ct["ccv"][:, i:i + 1])
                pz = psM.tile([128, SUB], F32, tag="pmm")
                mm(pz[:], ct["wz"][r0:r0 + 64, i * DI:(i + 1) * DI],
                   xnh[r0:r0 + 64, o + 3:o + 3 + SUB])
                nc.scalar.activation(szT[:, o:o + SUB], pz[:], AFT.Silu,
                                     bias=ct["cz"][:, i:i + 1])
                pxd = psA.tile([96, SUB], F32, tag="pbc")
                mm(pxd[:], ct["xpw"][:], xcT[:, o:o + SUB])
                xdbl = Tp.tile([96, SUB], F32, tag="scr")
                nc.scalar.copy(xdbl[:], pxd[:])
                pdt = psM.tile([128, SUB], F32, tag="pmm")
                mm(pdt[:], ct["dtw"][:], xdbl[0:4, :])
                # softplus(x) = x + ln(1 + exp(-x)); x = dt_raw + dt_bias
                xr = Tp.tile([128, SUB], F32, tag="spx")
                nc.scalar.activation(xr[:], pdt[:], AFT.Identity, bias=ct["dtb"][:])
                eneg = Tp.tile([128, SUB], F32, tag="spe")
                nc.scalar.activation(eneg[:], xr[:], AFT.Exp, scale=-1.0)
                lnv = Tp.tile([128, SUB], F32, tag="spl")
                nc.scalar.activation(lnv[:], eneg[:], AFT.Ln, bias=1.0)
                nc.vector.tensor_tensor(dtT[:, o:o + SUB], xr[:], lnv[:], AOT.add)
                nc.vector.tensor_tensor(dtuT[:, o:o + SUB], dtT[:, o:o + SUB],
                                        xcT[:, o:o + SUB], AOT.mult)
                pbb = psA.tile([128, SUB], F32, tag="pbc")
                mm(pbb[:], ct["patsbc"][32:48, 0:128], xdbl[32:48, :])
                nc.vector.tensor_copy(out=BbT[:, o:o + SUB], in_=pbb[:])
                pcb = psM.tile([128, SUB], F32, tag="pmm")
                mm(pcb[:], ct["patsbc"][64:80, 128:256], xdbl[64:80, :])
                nc.vector.tensor_copy(out=CbT[:, o:o + SUB], in_=pcb[:])

            # ---- selective scan over 16 (d-group) x 16 (state) partitions ----
            ySB = Sp.tile([128, TH], F32, tag="ySB")
            if STAGE <= 2:
                if i == 0:
                    nc.sync.dma_start(out=out[0:128, :], in_=dtT[:, 64:])
                    nc.sync.dma_start(out=out[128:256, :], in_=BbT[:, 64:])
                continue
            pY = psY.tile([128, TH], F32, tag="py")
            for g in range(16):
                hT = Cp.tile([128, SCT], F32, tag="hT", bufs=1)
                for c in range(4):
                    o = SUB * c
                    aT = Cp.tile([128, SUB], F32, tag="aT")
                    bT = Cp.tile([128, SUB], F32, tag="bT")
                    pda = psA.tile([128, SUB], F32, tag="pbc")
                    mm(pda[:], ct["patg"][:, 128 * g:128 * (g + 1)], dtT[:, o:o + SUB])
                    nc.scalar.activation(aT[:], pda[:], AFT.Exp,
                                         scale=ct["acols"][:, g:g + 1])
                    pdu = psM.tile([128, SUB], F32, tag="pmm")
                    mm(pdu[:], ct["patg"][:, 128 * g:128 * (g + 1)], dtuT[:, o:o + SUB])
                    nc.vector.scalar_tensor_tensor(bT[:], pdu[:], 1.0,
                                                   BbT[:, o:o + SUB],
                                                   AOT.mult, AOT.mult)
                    ini = 0.0 if c == 0 else hT[:, o - 1:o]
                    nc.vector.tensor_tensor_scan(hT[:, o:o + SUB], aT[:], bT[:],
                                                 ini, AOT.mult, AOT.add)
                for c in range(4):
                    o = OSUB * c
                    hcT = Tp.tile([128, OSUB], F32, tag="scr")
                    nc.vector.scalar_tensor_tensor(hcT[:], hT[:, 64 + o:64 + o + OSUB],
                                                   1.0, CbT[:, 64 + o:64 + o + OSUB],
                                                   AOT.mult, AOT.mult)
                    mm(pY[:, o:o + OSUB], ct["patyg"][:, 128 * g:128 * (g + 1)],
                       hcT[:], start=(g == 0), stop=(g == 15))
            for c in range(4):
                o = OSUB * c
                nc.scalar.copy(ySB[:, o:o + OSUB], pY[:, o:o + OSUB])

            if STAGE <= 3:
                if i == 0:
                    nc.sync.dma_start(out=out[0:128, :], in_=ySB[:])
                    nc.sync.dma_start(out=out[128:256, :], in_=CbT[:, 64:])
                continue
            # ---- gating, out_proj, LN1, MLP, skip (fused per subtile) ----
            mf_t = mfin[i // 2]
            for c in range(4):
                o = OSUB * c
                t5 = Tp.tile([128, OSUB], F32, tag="t5c")
                nc.vector.scalar_tensor_tensor(t5[:], xcT[:, 64 + o:64 + o + OSUB],
                                               ct["dp"][:], ySB[:, o:o + OSUB],
                                               AOT.mult, AOT.add)
                t6 = Tp.tile([128, OSUB], F32, tag="t6c")
                nc.vector.tensor_tensor(t6[:], t5[:], szT[:, 64 + o:64 + o + OSUB],
                                        AOT.mult)
                pm = psM.tile([64, OSUB], F32, tag="pmm")
                mm(pm[:], ct["opw"][:], t6[:])
                mSB = Tp.tile([64, OSUB], F32, tag="mSBc")
                nc.scalar.copy(mSB[:], pm[:])
                ps1 = psM.tile([1, OSUB], F32, tag="pmm")
                mm(ps1[:], ct["onesc"][0:64, :], mSB[:])
                s1 = Tp.tile([1, 512], F32, tag="rA")
                nc.vector.tensor_scalar(out=s1[:], in0=ps1[:],
                                        scalar1=1.0 / D, scalar2=None, op0=AOT.mult)
                sqt = Tp.tile([64, OSUB], F32, tag="scr")
                nc.scalar.activation(sqt[:], mSB[:], AFT.Square)
                pq1 = psM.tile([1, OSUB], F32, tag="pmm")
                mm(pq1[:], ct["onesc"][0:64, :], sqt[:])
                q1 = Tp.tile([1, 512], F32, tag="rB")
                nc.vector.tensor_scalar(out=q1[:], in0=pq1[:],
                                        scalar1=1.0 / D, scalar2=None, op0=AOT.mult)
                m2b = Tp.tile([1, 512], F32, tag="rC")
                nc.vector.tensor_tensor(m2b[:], s1[:], s1[:], AOT.mult)
                v1 = Tp.tile([1, 512], F32, tag="rD")
                nc.vector.tensor_tensor(v1[:], q1[:], m2b[:], AOT.subtract)
                sd1 = Tp.tile([1, 512], F32, tag="rC")
                nc.scalar.activation(sd1[:], v1[:], AFT.Sqrt, bias=eps_t[:])
                rs1 = Tp.tile([1, 512], F32, tag="rD")
                nc.vector.reciprocal_approx_fast(rs1[:], sd1[:])
                pmb1 = psA.tile([128, SUB], F32, tag="pbc")
                mm(pmb1[0:64, 0:OSUB], ct["ones1"][:, 0:64], s1[:])
                prb1 = psM.tile([128, SUB], F32, tag="pmm")
                mm(prb1[0:64, 0:OSUB], ct["ones1"][:, 0:64], rs1[:])
                tq = Tp.tile([64, OSUB], F32, tag="scr")
                nc.vector.scalar_tensor_tensor(tq[:], mSB[:], 1.0,
                                               pmb1[0:64, 0:OSUB], AOT.mult,
                                               AOT.subtract)
                mn = Tp.tile([64, OSUB], F32, tag="mnc")
                nc.vector.scalar_tensor_tensor(mn[:], tq[:], 1.0,
                                               prb1[0:64, 0:OSUB], AOT.mult, AOT.mult)
                ph1 = psM.tile([128, OSUB], F32, tag="pmm")
                mm(ph1[:], ct["fc1"][:, 0:128], mn[:])
                h1 = Tp.tile([128, OSUB], F32, tag="h1a")
                nc.scalar.activation(h1[:], ph1[:], AFT.Gelu, bias=ct["fc1b"][:, 0:1])
                ph2 = psM.tile([128, OSUB], F32, tag="pmm")
                mm(ph2[:], ct["fc1"][:, 128:256], mn[:])
                h2 = Tp.tile([128, OSUB], F32, tag="h1b")
                nc.scalar.activation(h2[:], ph2[:], AFT.Gelu, bias=ct["fc1b"][:, 1:2])
                pf2 = psM.tile([128, OSUB], F32, tag="pmm")
                mm(pf2[r0:r0 + 64, :], ct["fc2"][:, 0:64], h1[:],
                   start=True, stop=False)
                mm(pf2[r0:r0 + 64, :], ct["fc2"][:, 64:128], h2[:],
                   start=False, stop=True)
                tb = Tp.tile([128, OSUB], F32, tag="scr")
                nc.scalar.activation(tb[r0:r0 + 64, :], pf2[r0:r0 + 64, :],
                                     AFT.Identity, bias=ct["fc2b"][r0:r0 + 64, :])
                nc.vector.scalar_tensor_tensor(mf_t[r0:r0 + 64, o:o + OSUB],
                                               xnh[r0:r0 + 64, PAD + o:PAD + o + OSUB],
                                               ct["skips"][r0:r0 + 64, :],
                                               tb[r0:r0 + 64, :], AOT.mult, AOT.add)

        if STAGE == 4:
            for half in range(2):
                nc.sync.dma_start(out=out[128 * half:128 * (half + 1), :],
                                  in_=mfin[half][:])
        # ==== 1x1 conv across chunks + BN + SiLU ====
        for half in range(2 if STAGE >= 5 else 0):
            oSB = Sp.tile([128, TH], F32, tag="oSB")
            for c in range(4):
                o = OSUB * c
                pyc = psM.tile([128, OSUB], F32, tag="pmm")
                for t in range(2):
                    mm(pyc[:], ct["wout"][:, t * C + 128 * half:t * C + 128 * (half + 1)],
                       mfin[t][:, o:o + OSUB], start=(t == 0), stop=(t == 1))
                nc.scalar.activation(oSB[:, o:o + OSUB], pyc[:], AFT.Silu,
                                     scale=ct["bnsc"][:, half:half + 1],
                                     bias=ct["bnsh"][:, half:half + 1])
            nc.sync.dma_start(out=out[128 * half:128 * (half + 1), :], in_=oSB[:])

    nc.compile()
    _cache["nc"] = nc
    return nc


def _host_prep(inputs):
    f32 = np.float32

    def a(k):
        return np.asarray(inputs[k], f32)

    g, b_, Win = a("ln_g"), a("ln_b"), a("in_proj_w")
    convw, convb = a("conv_w"), a("conv_b")
    com = {}
    wctap = np.zeros((D, 16 * DI), f32)
    wz = np.zeros((D, 4 * DI), f32)
    ccv = np.zeros((DI, 4), f32)
    cz = np.zeros((DI, 4), f32)
    for i in range(4):
        gi, bi = g[64 * i:64 * (i + 1)], b_[64 * i:64 * (i + 1)]
        wxc = gi[:, None] * Win[:, :DI]
        for j in range(DC):
            wctap[:, (4 * i + j) * DI:(4 * i + j + 1) * DI] = wxc * convw[None, :, j]
        wz[:, i * DI:(i + 1) * DI] = gi[:, None] * Win[:, DI:]
        ccv[:, i] = (bi @ Win[:, :DI]) * convw.sum(1) + convb
        cz[:, i] = bi @ Win[:, DI:]
    com["wctap"], com["wz"] = np.tile(wctap, (2, 1)), np.tile(wz, (2, 1))
    com["ccv"], com["cz"] = ccv, cz
    xpw_raw = a("x_proj_w")
    xpw = np.zeros((DI, 96), f32)
    xpw[:, 0:DTR] = xpw_raw[:, 0:DTR]
    xpw[:, 32:48] = xpw_raw[:, DTR:DTR + DS]
    xpw[:, 64:80] = xpw_raw[:, DTR + DS:]
    com["xpw"] = xpw
    com["dtw"] = a("dt_proj_w")
    com["dtb"] = a("dt_proj_b").reshape(DI, 1)
    A = -np.exp(a("A_log"))
    acols = np.zeros((128, 16), f32)
    for p in range(128):
        for gg in range(16):
            acols[p, gg] = A[8 * gg + p // 16, p % 16]
    com["acols"] = acols
    com["dp"] = a("Dparam").reshape(DI, 1)
    com["opw"] = a("out_proj_w")
    g1, b1, fc1w = a("ln1_g"), a("ln1_b"), a("fc1_w")
    com["fc1"] = g1[:, None] * fc1w
    com["fc1b"] = (a("fc1_b") + b1 @ fc1w).reshape(2, 128).T.copy()
    fc2w = a("fc2_w")
    com["fc2"] = np.concatenate([fc2w[0:128, :], fc2w[128:256, :]], axis=1)
    com["fc2b"] = np.tile(a("fc2_b").reshape(64, 1), (2, 1))
    outcw = a("outc_w")
    wout = np.zeros((128, 2 * C), f32)
    for t in range(2):
        for i in (2 * t, 2 * t + 1):
            for d in range(D):
                wout[64 * (i % 2) + d, t * C:(t + 1) * C] = outcw[:, 4 * d + i]
    com["wout"] = wout
    sc = a("bn_g") / np.sqrt(a("bn_v") + EPS)
    com["bnsc"] = sc.reshape(2, 128).T.copy()
    com["bnsh"] = (a("bn_b") - a("bn_m") * sc).reshape(2, 128).T.copy()
    patg = np.zeros((128, 16 * 128), f32)
    patyg = np.zeros((128, 16 * 128), f32)
    for gg in range(16):
        for p in range(128):
            patg[8 * gg + p // 16, 128 * gg + p] = 1.0    # bcast d-row -> (d,s)
            patyg[p, 128 * gg + 8 * gg + p // 16] = 1.0   # sum over s -> d row
    patsbc = np.zeros((128, 256), f32)
    for p in range(128):
        patsbc[32 + p % 16, p] = 1.0          # B bcast lhsT rows 32:48
        patsbc[64 + p % 16, 128 + p] = 1.0    # C bcast lhsT rows 64:80
    com["patg"], com["patyg"], com["patsbc"] = patg, patyg, patsbc
    com["ones1"] = np.ones((1, 128), f32)
    com["onesc"] = np.ones((128, 1), f32)
    com["skips"] = np.full((128, 1), float(np.asarray(inputs["skip_scale"]).reshape(-1)[0]), f32)
    return {k: np.ascontiguousarray(v, f32) for k, v in com.items()}


def kernel(**inputs):
    nc = _build()
    com = _host_prep(inputs)
    x = np.asarray(inputs["x"], np.float32).reshape(B, C, N)
    in_maps = []
    for k in range(8):
        b, half = k // 2, k % 2
        if half == 0:
            xs = np.concatenate([np.zeros((C, PAD), np.float32), x[b, :, :TH]], axis=1)
        else:
            xs = x[b, :, TH - PAD:N]
        m = {"xs": np.ascontiguousarray(xs)}
        m.update(com)
        in_maps.append(m)
    res = run_bass_kernel_spmd(nc, in_maps, core_ids=list(range(8)))
    outp = np.zeros((B, C, N), np.float32)
    for k in range(8):
        b, half = k // 2, k % 2
        outp[b, :, half * TH:(half + 1) * TH] = res.results[k]["out"]
    return outp.reshape(B, C, H, W)



# revision 10
# speedup vs baseline: 1.2995x; 1.2995x over previous
"""Trainium2 Bass kernel for nn_CSI_75453985457421 (LN + chunked Mamba + MLP + 1x1conv + BN + SiLU).

Sharding: 8 cores = (batch b 0..3) x (time-half 0..1). Each core gets
x[b, :, half*2048-67 : half*2048+2048] (zero-padded before the sequence start)
and computes its 2048 output positions independently: 67 warmup columns
(3 causal-conv pad + 64 scan warmup; state decay <= exp(-0.68*64) << fp32 eps).

Device layout: time on the free axis. The selective scan runs with partitions
= (d_local, s): 16 groups of 8 d-channels x 16 states via the hardware
tensor_tensor_scan (DVE). dt/dtu/B/C broadcasts and the final sum over s are
TensorE pattern matmuls in float32r (1 cycle/row vs 4 for fp32); exp(A*dt) is
ScalarE with a per-partition scale. LN gamma/beta, the depthwise conv, the
channel interleave and BatchNorm are folded into weights on the host.
Stat broadcasts run on the idle GpSimd/Pool engine (partition_broadcast).
"""
import os
import sys

sys.path.insert(0, "/opt/trn_rl_repo")
STAGE = int(os.environ.get("KSTAGE", "9"))
import numpy as np
import concourse.bass as bass
import concourse.bacc as bacc
import concourse.tile as tile
from concourse import mybir
from concourse.bass_utils import run_bass_kernel_spmd

F32 = mybir.dt.float32
F32R = mybir.dt.float32r
AOT = mybir.AluOpType
AFT = mybir.ActivationFunctionType

B, C, H, W = 4, 256, 64, 64
N = H * W
D, DI, DS, DC, DTR, MH = 64, 128, 16, 4, 4, 256
EPS = 1e-5
PAD = 67
TH = 2048
TEXT = PAD + TH          # 2115
SCT = TEXT - 3           # 2112 = 4*528
SUB = 528
OSUB = 512

_cache = {}

# name -> (shape, is_matmul_operand)
_IN_SHAPES = dict(
    xs=((C, TEXT), True), wctap=((128, 16 * DI), True), wz=((128, 4 * DI), True),
    ccv=((DI, 4), False), cz=((DI, 4), False),
    xpw=((DI, 96), True), dtw=((DTR, DI), True), dtb=((DI, 1), False),
    acols=((128, 16), False), dp=((DI, 1), False), ndtb=((DI, 1), False),
    opw=((DI, D), True), fc1=((D, MH), True), fc1b=((128, 2), False),
    fc2=((128, 2 * D), True), fc2br=((1, D), True),
    wout=((128, 2 * C), True), bnsc=((128, 2), False), bnsh=((128, 2), False),
    patg=((128, 16 * 128), True), patyg=((128, 16 * 128), True),
    patsbc=((128, 256), True), onesr=((1, 512), True), onesc=((128, 1), True),
    skips=((128, 1), False),
)


def _build():
    if "nc" in _cache:
        return _cache["nc"]
    nc = bacc.Bacc("TRN2", target_bir_lowering=False, debug=False, num_devices=8)
    dram = {k: nc.dram_tensor(k, list(s), F32, kind="ExternalInput").ap()
            for k, (s, _) in _IN_SHAPES.items()}
    out = nc.dram_tensor("out", [C, TH], F32, kind="ExternalOutput").ap()

    with tile.TileContext(nc) as tc, \
            tc.tile_pool(name="const", bufs=1) as Kp, \
            tc.tile_pool(name="big", bufs=1) as Bp, \
            tc.tile_pool(name="seq", bufs=1) as Sp, \
            tc.tile_pool(name="tmp", bufs=2) as Tp, \
            tc.tile_pool(name="scan", bufs=3) as Cp, \
            tc.tile_pool(name="psA", bufs=1, space="PSUM") as psA, \
            tc.tile_pool(name="psM", bufs=1, space="PSUM") as psM, \
            tc.tile_pool(name="psY", bufs=1, space="PSUM") as psY:

        def mm(out_ap, lhsT, rhs, start=True, stop=True):
            n = out_ap.shape[-1]
            if n <= 512:
                nc.tensor.matmul(out_ap, lhsT, rhs, start=start, stop=stop)
                return
            o = 0
            while o < n:
                w_ = min(512, n - o)
                nc.tensor.matmul(out_ap[..., o:o + w_], lhsT, rhs[..., o:o + w_],
                                 start=start, stop=stop)
                o += w_

        ct = {}
        for k, (shp, is_r) in _IN_SHAPES.items():
            if k == "xs":
                continue
            dt_ = F32R if is_r else F32
            ct[k] = Kp.tile(list(shp), dt_, tag=k, name=f"ct_{k}")
            src = dram[k][:].bitcast(F32R) if is_r else dram[k][:]
            nc.sync.dma_start(out=ct[k][:], in_=src)
        eps_t = Kp.tile([1, 1], F32, tag="eps")
        nc.vector.memset(eps_t[:], EPS)

        xh = [Bp.tile([128, TEXT], F32R, tag=f"xh{h}", name=f"xh{h}") for h in range(2)]
        for h in range(2):
            nc.sync.dma_start(out=xh[h][:], in_=dram["xs"][128 * h:128 * (h + 1), :].bitcast(F32R))

        # ---- LayerNorm over C: fused per-subtile stats + apply ----
        # last subtile overlaps col 2047 so every width stays even (fp32r
        # matmul requires an even moving width); re-normalizing an already
        # normalized column is a ~eps no-op.
        nsub = [(0, 512), (512, 512), (1024, 512), (1536, 512), (TEXT - 68, 68)]
        for (o, w_) in nsub:
            pse = psM.tile([1, 512], F32, tag="pmm")
            for h in range(2):
                mm(pse[:, :w_], ct["onesc"][:], xh[h][:, o:o + w_],
                   start=(h == 0), stop=(h == 1))
            mean = Tp.tile([1, 512], F32, tag="rA", bufs=1)
            nc.vector.tensor_scalar(out=mean[:, :w_], in0=pse[:, :w_],
                                    scalar1=1.0 / C, scalar2=None, op0=AOT.mult)
            psq = psM.tile([1, 512], F32, tag="pmm")
            for h in range(2):
                sqt = Tp.tile([128, 512], F32R, tag="scrR")
                nc.scalar.activation(sqt[:, :w_], xh[h][:, o:o + w_].bitcast(F32),
                                     AFT.Square)
                mm(psq[:, :w_], ct["onesc"][:], sqt[:, :w_],
                   start=(h == 0), stop=(h == 1))
            sqm = Tp.tile([1, 512], F32, tag="rB", bufs=1)
            nc.vector.tensor_scalar(out=sqm[:, :w_], in0=psq[:, :w_],
                                    scalar1=1.0 / C, scalar2=None, op0=AOT.mult)
            m2 = Tp.tile([1, 512], F32, tag="rC", bufs=1)
            nc.vector.tensor_tensor(m2[:, :w_], mean[:, :w_], mean[:, :w_], AOT.mult)
            var = Tp.tile([1, 512], F32, tag="rD", bufs=1)
            nc.vector.tensor_tensor(var[:, :w_], sqm[:, :w_], m2[:, :w_], AOT.subtract)
            sd = Tp.tile([1, 512], F32, tag="rC", bufs=1)
            nc.scalar.activation(sd[:, :w_], var[:, :w_], AFT.Sqrt, bias=eps_t[:])
            rstd = Tp.tile([1, 512], F32, tag="rD", bufs=1)
            nc.vector.reciprocal_approx_fast(rstd[:, :w_], sd[:, :w_])
            bmean = Tp.tile([128, 512], F32, tag="bcA", bufs=1)
            nc.gpsimd.partition_broadcast(bmean[:, :w_], mean[:, :w_])
            brstd = Tp.tile([128, 512], F32, tag="bcB", bufs=1)
            nc.gpsimd.partition_broadcast(brstd[:, :w_], rstd[:, :w_])
            for h in range(2):
                tmp = Tp.tile([128, 512], F32, tag="scr")
                nc.vector.scalar_tensor_tensor(tmp[:, :w_], xh[h][:, o:o + w_].bitcast(F32),
                                               1.0, bmean[:, :w_], AOT.mult, AOT.subtract)
                nc.vector.scalar_tensor_tensor(xh[h][:, o:o + w_], tmp[:, :w_], 1.0,
                                               brstd[:, :w_], AOT.mult, AOT.mult)

        mfin = [Bp.tile([128, TH], F32R, tag=f"mfin{h}", name=f"mfin{h}") for h in range(2)]
        if STAGE <= 1:
            for half in range(2):
                nc.sync.dma_start(out=out[128 * half:128 * (half + 1), :],
                                  in_=xh[half][:, PAD:].bitcast(F32))
        nseq = 0 if STAGE <= 1 else 4
        # ==== per sequence (channel chunk) ====
        for i in range(nseq):
            xnh = xh[i // 2]
            r0 = 64 * (i % 2)
            xcT = Sp.tile([128, SCT], F32R, tag="xcT")
            szT = Sp.tile([128, SCT], F32, tag="szT")
            dtT = Sp.tile([128, SCT], F32R, tag="dtT")
            dtuT = Sp.tile([128, SCT], F32R, tag="dtuT")
            BbT = Sp.tile([128, SCT], F32, tag="BbT")
            CbT = Sp.tile([128, SCT], F32, tag="CbT")

            for c in range(4):
                o = SUB * c
                pxt = psA.tile([128, SUB], F32, tag="pbc")
                for j in range(DC):
                    mm(pxt[:], ct["wctap"][r0:r0 + 64, (4 * i + j) * DI:(4 * i + j + 1) * DI],
                       xnh[r0:r0 + 64, o + j:o + j + SUB],
                       start=(j == 0), stop=(j == DC - 1))
                nc.scalar.activation(xcT[:, o:o + SUB], pxt[:], AFT.Silu,
                                     bias=ct["ccv"][:, i:i + 1])
                pz = psM.tile([128, SUB], F32, tag="pmm")
                mm(pz[:], ct["wz"][r0:r0 + 64, i * DI:(i + 1) * DI],
                   xnh[r0:r0 + 64, o + 3:o + 3 + SUB])
                nc.scalar.activation(szT[:, o:o + SUB], pz[:], AFT.Silu,
                                     bias=ct["cz"][:, i:i + 1])
                pxd = psA.tile([96, SUB], F32, tag="pbc")
                mm(pxd[:], ct["xpw"][:], xcT[:, o:o + SUB])
                xdbl = Tp.tile([96, SUB], F32R, tag="xdblR")
                nc.scalar.copy(xdbl[:], pxd[:])
                pdt = psM.tile([128, SUB], F32, tag="pmm")
                mm(pdt[:], ct["dtw"][:], xdbl[0:4, :])
                # softplus(x) = x + ln(1 + exp(-x)); x = pdt + dtb
                eneg = Tp.tile([128, SUB], F32, tag="spe")
                nc.scalar.activation(eneg[:], pdt[:], AFT.Exp, scale=-1.0,
                                     bias=ct["ndtb"][:])
                lnv = Tp.tile([128, SUB], F32, tag="spl")
                nc.scalar.activation(lnv[:], eneg[:], AFT.Ln, bias=1.0)
                nc.vector.scalar_tensor_tensor(dtT[:, o:o + SUB], pdt[:],
                                               ct["dtb"][:], lnv[:],
                                               AOT.add, AOT.add)
                nc.vector.tensor_tensor(dtuT[:, o:o + SUB], dtT[:, o:o + SUB].bitcast(F32),
                                        xcT[:, o:o + SUB].bitcast(F32), AOT.mult)
                pbb = psA.tile([128, SUB], F32, tag="pbc")
                mm(pbb[:], ct["patsbc"][32:48, 0:128], xdbl[32:48, :])
                nc.scalar.copy(BbT[:, o:o + SUB], pbb[:])
                pcb = psM.tile([128, SUB], F32, tag="pmm")
                mm(pcb[:], ct["patsbc"][64:80, 128:256], xdbl[64:80, :])
                nc.scalar.copy(CbT[:, o:o + SUB], pcb[:])

            # ---- selective scan over 16 (d-group) x 16 (state) partitions ----
            if STAGE <= 2:
                if i == 0:
                    nc.sync.dma_start(out=out[0:128, :], in_=dtT[:, 64:].bitcast(F32))
                    nc.sync.dma_start(out=out[128:256, :], in_=BbT[:, 64:])
                continue
            pY = psY.tile([128, TH], F32, tag="py")
            for g in range(16):
                hT = Cp.tile([128, SCT], F32, tag="hT", bufs=1)
                for c in range(4):
                    o = SUB * c
                    aT = Cp.tile([128, SUB], F32, tag="aT")
                    bT = Cp.tile([128, SUB], F32, tag="bT")
                    pda = psA.tile([128, SUB], F32, tag="pbc")
                    mm(pda[:], ct["patg"][:, 128 * g:128 * (g + 1)], dtT[:, o:o + SUB])
                    nc.scalar.activation(aT[:], pda[:], AFT.Exp,
                                         scale=ct["acols"][:, g:g + 1])
                    pdu = psM.tile([128, SUB], F32, tag="pmm")
                    mm(pdu[:], ct["patg"][:, 128 * g:128 * (g + 1)], dtuT[:, o:o + SUB])
                    nc.vector.scalar_tensor_tensor(bT[:], pdu[:], 1.0,
                                                   BbT[:, o:o + SUB],
                                                   AOT.mult, AOT.mult)
                    ini = 0.0 if c == 0 else hT[:, o - 1:o]
                    nc.vector.tensor_tensor_scan(hT[:, o:o + SUB], aT[:], bT[:],
                                                 ini, AOT.mult, AOT.add)
                for c in range(4):
                    o = OSUB * c
                    hcT = Tp.tile([128, OSUB], F32R, tag="hcR")
                    nc.vector.scalar_tensor_tensor(hcT[:], hT[:, 64 + o:64 + o + OSUB],
                                                   1.0, CbT[:, 64 + o:64 + o + OSUB],
                                                   AOT.mult, AOT.mult)
                    mm(pY[:, o:o + OSUB], ct["patyg"][:, 128 * g:128 * (g + 1)],
                       hcT[:], start=(g == 0), stop=(g == 15))

            if STAGE <= 3:
                if i == 0:
                    ySB = Sp.tile([128, TH], F32, tag="oSB")
                    for c in range(4):
                        o = OSUB * c
                        nc.scalar.copy(ySB[:, o:o + OSUB], pY[:, o:o + OSUB])
                    nc.sync.dma_start(out=out[0:128, :], in_=ySB[:])
                    nc.sync.dma_start(out=out[128:256, :], in_=CbT[:, 64:])
                continue
            # ---- gating, out_proj, LN1, MLP, skip (fused per subtile) ----
            mf_t = mfin[i // 2]
            for c in range(4):
                o = OSUB * c
                t5 = Tp.tile([128, OSUB], F32, tag="t5c")
                nc.vector.scalar_tensor_tensor(t5[:], xcT[:, 64 + o:64 + o + OSUB].bitcast(F32),
                                               ct["dp"][:], pY[:, o:o + OSUB],
                                               AOT.mult, AOT.add)
                t6 = Tp.tile([128, OSUB], F32R, tag="t6c")
                nc.vector.tensor_tensor(t6[:], t5[:], szT[:, 64 + o:64 + o + OSUB],
                                        AOT.mult)
                pm = psM.tile([64, OSUB], F32, tag="pmm")
                mm(pm[:], ct["opw"][:], t6[:])
                mSB = Tp.tile([64, OSUB], F32R, tag="mSBc")
                nc.scalar.copy(mSB[:], pm[:])
                ps1 = psM.tile([1, OSUB], F32, tag="pmm")
                mm(ps1[:], ct["onesc"][0:64, :], mSB[:])
                s1 = Tp.tile([1, 512], F32, tag="rA", bufs=1)
                nc.vector.tensor_scalar(out=s1[:], in0=ps1[:],
                                        scalar1=1.0 / D, scalar2=None, op0=AOT.mult)
                sqt = Tp.tile([64, OSUB], F32R, tag="scrR")
                nc.scalar.activation(sqt[:], mSB[:].bitcast(F32), AFT.Square)
                pq1 = psM.tile([1, OSUB], F32, tag="pmm")
                mm(pq1[:], ct["onesc"][0:64, :], sqt[:])
                q1 = Tp.tile([1, 512], F32, tag="rB", bufs=1)
                nc.vector.tensor_scalar(out=q1[:], in0=pq1[:],
                                        scalar1=1.0 / D, scalar2=None, op0=AOT.mult)
                m2b = Tp.tile([1, 512], F32, tag="rC", bufs=1)
                nc.vector.tensor_tensor(m2b[:], s1[:], s1[:], AOT.mult)
                v1 = Tp.tile([1, 512], F32, tag="rD", bufs=1)
                nc.vector.tensor_tensor(v1[:], q1[:], m2b[:], AOT.subtract)
                sd1 = Tp.tile([1, 512], F32, tag="rC", bufs=1)
                nc.scalar.activation(sd1[:], v1[:], AFT.Sqrt, bias=eps_t[:])
                rs1 = Tp.tile([1, 512], F32, tag="rD", bufs=1)
                nc.vector.reciprocal_approx_fast(rs1[:], sd1[:])
                bmn = Tp.tile([64, OSUB], F32, tag="bcA", bufs=1)
                nc.gpsimd.partition_broadcast(bmn[:], s1[:])
                brs = Tp.tile([64, OSUB], F32, tag="bcB", bufs=1)
                nc.gpsimd.partition_broadcast(brs[:], rs1[:])
                tq = Tp.tile([64, OSUB], F32, tag="scr")
                nc.vector.scalar_tensor_tensor(tq[:], mSB[:].bitcast(F32), 1.0,
                                               bmn[:], AOT.mult, AOT.subtract)
                mn = Tp.tile([64, OSUB], F32R, tag="mnc")
                nc.vector.scalar_tensor_tensor(mn[:], tq[:], 1.0,
                                               brs[:], AOT.mult, AOT.mult)
                ph1 = psM.tile([128, OSUB], F32, tag="pmm")
                mm(ph1[:], ct["fc1"][:, 0:128], mn[:])
                h1 = Tp.tile([128, OSUB], F32R, tag="h1a")
                nc.scalar.activation(h1[:], ph1[:], AFT.Gelu, bias=ct["fc1b"][:, 0:1])
                ph2 = psM.tile([128, OSUB], F32, tag="pmm")
                mm(ph2[:], ct["fc1"][:, 128:256], mn[:])
                h2 = Tp.tile([128, OSUB], F32R, tag="h1b")
                nc.scalar.activation(h2[:], ph2[:], AFT.Gelu, bias=ct["fc1b"][:, 1:2])
                pf2 = psM.tile([64, OSUB], F32, tag="pmm")
                mm(pf2[:], ct["fc2"][:, 0:64], h1[:],
                   start=True, stop=False)
                mm(pf2[:], ct["fc2"][:, 64:128], h2[:],
                   start=False, stop=False)
                mm(pf2[:], ct["fc2br"][:], ct["onesr"][:],
                   start=False, stop=True)
                nc.vector.scalar_tensor_tensor(mf_t[r0:r0 + 64, o:o + OSUB],
                                               xnh[r0:r0 + 64, PAD + o:PAD + o + OSUB].bitcast(F32),
                                               ct["skips"][r0:r0 + 64, :],
                                               pf2[:], AOT.mult, AOT.add)

        if STAGE == 4:
            for half in range(2):
                nc.sync.dma_start(out=out[128 * half:128 * (half + 1), :],
                                  in_=mfin[half][:].bitcast(F32))
        # ==== 1x1 conv across chunks + BN + SiLU ====
        for half in range(2 if STAGE >= 5 else 0):
            oSB = Sp.tile([128, TH], F32, tag="oSB")
            for c in range(4):
                o = OSUB * c
                pyc = psM.tile([128, OSUB], F32, tag="pmm")
                for t in range(2):
                    mm(pyc[:], ct["wout"][:, t * C + 128 * half:t * C + 128 * (half + 1)],
                       mfin[t][:, o:o + OSUB], start=(t == 0), stop=(t == 1))
                nc.scalar.activation(oSB[:, o:o + OSUB], pyc[:], AFT.Silu,
                                     scale=ct["bnsc"][:, half:half + 1],
                                     bias=ct["bnsh"][:, half:half + 1])
            nc.sync.dma_start(out=out[128 * half:128 * (half + 1), :], in_=oSB[:])

    nc.compile()
    _cache["nc"] = nc
    return nc


def _host_prep(inputs):
    f32 = np.float32

    def a(k):
        return np.asarray(inputs[k], f32)

    g, b_, Win = a("ln_g"), a("ln_b"), a("in_proj_w")
    convw, convb = a("conv_w"), a("conv_b")
    com = {}
    wctap = np.zeros((D, 16 * DI), f32)
    wz = np.zeros((D, 4 * DI), f32)
    ccv = np.zeros((DI, 4), f32)
    cz = np.zeros((DI, 4), f32)
    for i in range(4):
        gi, bi = g[64 * i:64 * (i + 1)], b_[64 * i:64 * (i + 1)]
        wxc = gi[:, None] * Win[:, :DI]
        for j in range(DC):
            wctap[:, (4 * i + j) * DI:(4 * i + j + 1) * DI] = wxc * convw[None, :, j]
        wz[:, i * DI:(i + 1) * DI] = gi[:, None] * Win[:, DI:]
        ccv[:, i] = (bi @ Win[:, :DI]) * convw.sum(1) + convb
        cz[:, i] = bi @ Win[:, DI:]
    com["wctap"], com["wz"] = np.tile(wctap, (2, 1)), np.tile(wz, (2, 1))
    com["ccv"], com["cz"] = ccv, cz
    xpw_raw = a("x_proj_w")
    xpw = np.zeros((DI, 96), f32)
    xpw[:, 0:DTR] = xpw_raw[:, 0:DTR]
    xpw[:, 32:48] = xpw_raw[:, DTR:DTR + DS]
    xpw[:, 64:80] = xpw_raw[:, DTR + DS:]
    com["xpw"] = xpw
    com["dtw"] = a("dt_proj_w")
    com["dtb"] = a("dt_proj_b").reshape(DI, 1)
    com["ndtb"] = -a("dt_proj_b").reshape(DI, 1)
    A = -np.exp(a("A_log"))
    acols = np.zeros((128, 16), f32)
    for p in range(128):
        for gg in range(16):
            acols[p, gg] = A[8 * gg + p // 16, p % 16]
    com["acols"] = acols
    com["dp"] = a("Dparam").reshape(DI, 1)
    com["opw"] = a("out_proj_w")
    g1, b1, fc1w = a("ln1_g"), a("ln1_b"), a("fc1_w")
    com["fc1"] = g1[:, None] * fc1w
    com["fc1b"] = (a("fc1_b") + b1 @ fc1w).reshape(2, 128).T.copy()
    fc2w = a("fc2_w")
    com["fc2"] = np.concatenate([fc2w[0:128, :], fc2w[128:256, :]], axis=1)
    com["fc2br"] = a("fc2_b").reshape(1, D)
    outcw = a("outc_w")
    wout = np.zeros((128, 2 * C), f32)
    for t in range(2):
        for i in (2 * t, 2 * t + 1):
            for d in range(D):
                wout[64 * (i % 2) + d, t * C:(t + 1) * C] = outcw[:, 4 * d + i]
    com["wout"] = wout
    sc = a("bn_g") / np.sqrt(a("bn_v") + EPS)
    com["bnsc"] = sc.reshape(2, 128).T.copy()
    com["bnsh"] = (a("bn_b") - a("bn_m") * sc).reshape(2, 128).T.copy()
    patg = np.zeros((128, 16 * 128), f32)
    patyg = np.zeros((128, 16 * 128), f32)
    for gg in range(16):
        for p in range(128):
            patg[8 * gg + p // 16, 128 * gg + p] = 1.0    # bcast d-row -> (d,s)
            patyg[p, 128 * gg + 8 * gg + p // 16] = 1.0   # sum over s -> d row
    patsbc = np.zeros((128, 256), f32)
    for p in range(128):
        patsbc[32 + p % 16, p] = 1.0          # B bcast lhsT rows 32:48
        patsbc[64 + p % 16, 128 + p] = 1.0    # C bcast lhsT rows 64:80
    com["patg"], com["patyg"], com["patsbc"] = patg, patyg, patsbc
    com["onesr"] = np.ones((1, 512), f32)
    com["onesc"] = np.ones((128, 1), f32)
    com["skips"] = np.full((128, 1), float(np.asarray(inputs["skip_scale"]).reshape(-1)[0]), f32)
    return {k: np.ascontiguousarray(v, f32) for k, v in com.items()}


def kernel(**inputs):
    nc = _build()
    com = _host_prep(inputs)
    x = np.asarray(inputs["x"], np.float32).reshape(B, C, N)
    in_maps = []
    for k in range(8):
        b, half = k // 2, k % 2
        if half == 0:
            xs = np.concatenate([np.zeros((C, PAD), np.float32), x[b, :, :TH]], axis=1)
        else:
            xs = x[b, :, TH - PAD:N]
        m = {"xs": np.ascontiguousarray(xs)}
        m.update(com)
        in_maps.append(m)
    res = run_bass_kernel_spmd(nc, in_maps, core_ids=list(range(8)))
    outp = np.zeros((B, C, N), np.float32)
    for k in range(8):
        b, half = k // 2, k % 2
        outp[b, :, half * TH:(half + 1) * TH] = res.results[k]["out"]
    return outp.reshape(B, C, H, W)


# revision 15
# speedup vs baseline: 1.3181x; 1.0143x over previous
"""Trainium2 Bass kernel for nn_CSI_75453985457421 (LN + chunked Mamba + MLP + 1x1conv + BN + SiLU).

Sharding: 8 cores = (batch b 0..3) x (time-half 0..1). Each core gets
x[b, :, half*2048-67 : half*2048+2048] (zero-padded before the sequence start)
and computes its 2048 output positions independently: 67 warmup columns
(3 causal-conv pad + 64 scan warmup; state decay <= exp(-0.68*64) << fp32 eps).

Device layout: time on the free axis. The selective scan runs with partitions
= (d_local, s): 16 groups of 8 d-channels x 16 states via the hardware
tensor_tensor_scan (DVE). dt/dtu/B/C broadcasts and the final sum over s are
TensorE pattern matmuls in float32r (1 cycle/row vs 4 for fp32); exp(A*dt) is
ScalarE with a per-partition scale. LN gamma/beta, the depthwise conv, the
channel interleave and BatchNorm are folded into weights on the host.
Stat broadcasts run on the idle GpSimd/Pool engine (partition_broadcast).
"""
import os
import sys

sys.path.insert(0, "/opt/trn_rl_repo")
STAGE = int(os.environ.get("KSTAGE", "9"))
import numpy as np
import concourse.bass as bass
import concourse.bacc as bacc
import concourse.tile as tile
from concourse import mybir
from concourse.bass_utils import run_bass_kernel_spmd

F32 = mybir.dt.float32
F32R = mybir.dt.float32r
AOT = mybir.AluOpType
AFT = mybir.ActivationFunctionType

B, C, H, W = 4, 256, 64, 64
N = H * W
D, DI, DS, DC, DTR, MH = 64, 128, 16, 4, 4, 256
EPS = 1e-5
PAD = 67
TH = 2048
TEXT = PAD + TH          # 2115
SCT = TEXT - 3           # 2112 = 4*528
SUB = 528
OSUB = 512

_cache = {}

# name -> (shape, is_matmul_operand)
_IN_SHAPES = dict(
    xs=((C, TEXT), True), wctap=((128, 16 * DI), True), wz=((128, 4 * DI), True),
    ccv=((DI, 4), False), cz=((DI, 4), False),
    xpw=((DI, 96), True), dtw=((DTR, DI), True), dtb=((DI, 1), False),
    acols=((128, 16), False), dp=((DI, 1), False), ndtb=((DI, 1), False),
    opw=((DI, D), True), fc1=((D, MH), True), fc1b=((128, 2), False),
    fc2=((128, 2 * D), True), fc2br=((1, D), True),
    wout=((128, 2 * C), True), bnsc=((128, 2), False), bnsh=((128, 2), False),
    patg=((128, 16 * 128), True), patyg=((128, 16 * 128), True),
    patsbc=((128, 256), True), onesr=((1, 512), True), onesc=((128, 1), True),
    skips=((128, 1), False),
)


def _build():
    if "nc" in _cache:
        return _cache["nc"]
    nc = bacc.Bacc("TRN2", target_bir_lowering=False, debug=False, num_devices=8)
    dram = {k: nc.dram_tensor(k, list(s), F32, kind="ExternalInput").ap()
            for k, (s, _) in _IN_SHAPES.items()}
    out = nc.dram_tensor("out", [C, TH], F32, kind="ExternalOutput").ap()

    with tile.TileContext(nc) as tc, \
            tc.tile_pool(name="const", bufs=1) as Kp, \
            tc.tile_pool(name="big", bufs=1) as Bp, \
            tc.tile_pool(name="seq", bufs=1) as Sp, \
            tc.tile_pool(name="tmp", bufs=2) as Tp, \
            tc.tile_pool(name="scan", bufs=2) as Cp, \
            tc.tile_pool(name="psA", bufs=1, space="PSUM") as psA, \
            tc.tile_pool(name="psM", bufs=1, space="PSUM") as psM, \
            tc.tile_pool(name="psY", bufs=1, space="PSUM") as psY:

        def mm(out_ap, lhsT, rhs, start=True, stop=True):
            n = out_ap.shape[-1]
            if n <= 512:
                nc.tensor.matmul(out_ap, lhsT, rhs, start=start, stop=stop)
                return
            o = 0
            while o < n:
                w_ = min(512, n - o)
                nc.tensor.matmul(out_ap[..., o:o + w_], lhsT, rhs[..., o:o + w_],
                                 start=start, stop=stop)
                o += w_

        ct = {}
        for k, (shp, is_r) in _IN_SHAPES.items():
            if k == "xs":
                continue
            dt_ = F32R if is_r else F32
            ct[k] = Kp.tile(list(shp), dt_, tag=k, name=f"ct_{k}")
            src = dram[k][:].bitcast(F32R) if is_r else dram[k][:]
            nc.sync.dma_start(out=ct[k][:], in_=src)
        eps_t = Kp.tile([1, 1], F32, tag="eps")
        nc.vector.memset(eps_t[:], EPS)

        xh = [Bp.tile([128, TEXT], F32R, tag=f"xh{h}", name=f"xh{h}") for h in range(2)]
        for h in range(2):
            nc.sync.dma_start(out=xh[h][:], in_=dram["xs"][128 * h:128 * (h + 1), :].bitcast(F32R))

        # ---- LayerNorm over C: fused per-subtile stats + apply ----
        # last subtile overlaps col 2047 so every width stays even (fp32r
        # matmul requires an even moving width); re-normalizing an already
        # normalized column is a ~eps no-op.
        nsub = [(0, 512), (512, 512), (1024, 512), (1536, 512), (TEXT - 68, 68)]
        for (o, w_) in nsub:
            pse = psM.tile([1, 512], F32, tag="pmm")
            for h in range(2):
                mm(pse[:, :w_], ct["onesc"][:], xh[h][:, o:o + w_],
                   start=(h == 0), stop=(h == 1))
            mean = Tp.tile([1, 512], F32, tag="rA", bufs=1)
            nc.vector.tensor_scalar(out=mean[:, :w_], in0=pse[:, :w_],
                                    scalar1=1.0 / C, scalar2=None, op0=AOT.mult)
            psq = psM.tile([1, 512], F32, tag="pmm")
            for h in range(2):
                sqt = Tp.tile([128, 512], F32R, tag="scrR", bufs=1)
                nc.scalar.activation(sqt[:, :w_], xh[h][:, o:o + w_].bitcast(F32),
                                     AFT.Square)
                mm(psq[:, :w_], ct["onesc"][:], sqt[:, :w_],
                   start=(h == 0), stop=(h == 1))
            sqm = Tp.tile([1, 512], F32, tag="rB", bufs=1)
            nc.vector.tensor_scalar(out=sqm[:, :w_], in0=psq[:, :w_],
                                    scalar1=1.0 / C, scalar2=None, op0=AOT.mult)
            m2 = Tp.tile([1, 512], F32, tag="rC", bufs=1)
            nc.vector.tensor_tensor(m2[:, :w_], mean[:, :w_], mean[:, :w_], AOT.mult)
            var = Tp.tile([1, 512], F32, tag="rD", bufs=1)
            nc.vector.tensor_tensor(var[:, :w_], sqm[:, :w_], m2[:, :w_], AOT.subtract)
            sd = Tp.tile([1, 512], F32, tag="rC", bufs=1)
            nc.scalar.activation(sd[:, :w_], var[:, :w_], AFT.Sqrt, bias=eps_t[:])
            rstd = Tp.tile([1, 512], F32, tag="rD", bufs=1)
            nc.vector.reciprocal_approx_fast(rstd[:, :w_], sd[:, :w_])
            bmean = Tp.tile([128, 512], F32, tag="bcA", bufs=1)
            nc.gpsimd.partition_broadcast(bmean[:, :w_], mean[:, :w_])
            brstd = Tp.tile([128, 512], F32, tag="bcB", bufs=1)
            nc.gpsimd.partition_broadcast(brstd[:, :w_], rstd[:, :w_])
            for h in range(2):
                tmp = Tp.tile([128, 512], F32, tag="scr")
                nc.vector.scalar_tensor_tensor(tmp[:, :w_], xh[h][:, o:o + w_].bitcast(F32),
                                               1.0, bmean[:, :w_], AOT.mult, AOT.subtract)
                nc.vector.scalar_tensor_tensor(xh[h][:, o:o + w_], tmp[:, :w_], 1.0,
                                               brstd[:, :w_], AOT.mult, AOT.mult)

        mfin = [Bp.tile([128, TH], F32R, tag=f"mfin{h}", name=f"mfin{h}") for h in range(2)]
        if STAGE <= 1:
            for half in range(2):
                nc.sync.dma_start(out=out[128 * half:128 * (half + 1), :],
                                  in_=xh[half][:, PAD:].bitcast(F32))
        nseq = 0 if STAGE <= 1 else 4
        # ==== per sequence (channel chunk) ====
        # Emission is batched by activation-table set to minimize 1.3us
        # LoadActFuncSet switches: pass1 Silu/Copy (set 18), pass2+scan
        # Exp/Ln (set 6), MLP stats Square/Sqrt (set 3), Gelu (set 10).
        for i in range(nseq):
            xnh = xh[i // 2]
            r0 = 64 * (i % 2)
            xcT = Sp.tile([128, SCT], F32R, tag="xcT")
            szT = Sp.tile([128, SCT], mybir.dt.bfloat16, tag="szT")
            dtT = Sp.tile([128, SCT], F32R, tag="dtT")
            dtuT = Sp.tile([128, SCT], F32R, tag="dtuT")
            BbT = Sp.tile([128, SCT], F32, tag="BbT")
            CbT = Sp.tile([128, SCT], F32, tag="CbT")
            xdblT = Sp.tile([96, SCT], F32R, tag="xdblT")

            # -- pass 1: projections + Silu (act set 18) + PSUM copies --
            for c in range(4):
                o = SUB * c
                pxt = psA.tile([128, SUB], F32, tag="pbc")
                for j in range(DC):
                    mm(pxt[:], ct["wctap"][r0:r0 + 64, (4 * i + j) * DI:(4 * i + j + 1) * DI],
                       xnh[r0:r0 + 64, o + j:o + j + SUB],
                       start=(j == 0), stop=(j == DC - 1))
                nc.scalar.activation(xcT[:, o:o + SUB], pxt[:], AFT.Silu,
                                     bias=ct["ccv"][:, i:i + 1])
                pz = psM.tile([128, SUB], F32, tag="pmm")
                mm(pz[:], ct["wz"][r0:r0 + 64, i * DI:(i + 1) * DI],
                   xnh[r0:r0 + 64, o + 3:o + 3 + SUB])
                nc.scalar.activation(szT[:, o:o + SUB], pz[:], AFT.Silu,
                                     bias=ct["cz"][:, i:i + 1])
                pxd = psA.tile([96, SUB], F32, tag="pbc")
                mm(pxd[:], ct["xpw"][:], xcT[:, o:o + SUB])
                nc.scalar.copy(xdblT[:, o:o + SUB], pxd[:])
                pbb = psA.tile([128, SUB], F32, tag="pbc")
                mm(pbb[:], ct["patsbc"][32:48, 0:128], xdblT[32:48, o:o + SUB])
                nc.scalar.copy(BbT[:, o:o + SUB], pbb[:])
                pcb = psM.tile([128, SUB], F32, tag="pmm")
                mm(pcb[:], ct["patsbc"][64:80, 128:256], xdblT[64:80, o:o + SUB])
                nc.scalar.copy(CbT[:, o:o + SUB], pcb[:])

            # -- pass 2: dt softplus (act set 6: Exp/Ln) + dtu on Pool --
            for c in range(4):
                o = SUB * c
                pdt = psM.tile([128, SUB], F32, tag="pmm")
                mm(pdt[:], ct["dtw"][:], xdblT[0:4, o:o + SUB])
                # softplus(x) = x + ln(1 + exp(-x)); x = pdt + dtb
                eneg = Tp.tile([128, SUB], F32, tag="spe", bufs=1)
                nc.scalar.activation(eneg[:], pdt[:], AFT.Exp, scale=-1.0,
                                     bias=ct["ndtb"][:])
                lnv = Tp.tile([128, SUB], F32, tag="spl", bufs=1)
                nc.scalar.activation(lnv[:], eneg[:], AFT.Ln, bias=1.0)
                nc.vector.scalar_tensor_tensor(dtT[:, o:o + SUB], pdt[:],
                                               ct["dtb"][:], lnv[:],
                                               AOT.add, AOT.add)
                nc.gpsimd.tensor_tensor(dtuT[:, o:o + SUB], dtT[:, o:o + SUB].bitcast(F32),
                                        xcT[:, o:o + SUB].bitcast(F32), AOT.mult)

            # ---- selective scan over 16 (d-group) x 16 (state) partitions ----
            if STAGE <= 2:
                if i == 0:
                    nc.sync.dma_start(out=out[0:128, :], in_=dtT[:, 64:].bitcast(F32))
                    nc.sync.dma_start(out=out[128:256, :], in_=BbT[:, 64:])
                continue
            pY = psY.tile([128, TH], F32, tag="py")
            for g in range(16):
                hT = Cp.tile([128, SCT], F32, tag="hT", bufs=1)
                for c in range(4):
                    o = SUB * c
                    aT = Cp.tile([128, SUB], F32, tag="aT")
                    bT = Cp.tile([128, SUB], F32, tag="bT")
                    pda = psA.tile([128, SUB], F32, tag="pbc")
                    mm(pda[:], ct["patg"][:, 128 * g:128 * (g + 1)], dtT[:, o:o + SUB])
                    nc.scalar.activation(aT[:], pda[:], AFT.Exp,
                                         scale=ct["acols"][:, g:g + 1])
                    pdu = psM.tile([128, SUB], F32, tag="pmm")
                    mm(pdu[:], ct["patg"][:, 128 * g:128 * (g + 1)], dtuT[:, o:o + SUB])
                    nc.vector.scalar_tensor_tensor(bT[:], pdu[:], 1.0,
                                                   BbT[:, o:o + SUB],
                                                   AOT.mult, AOT.mult)
                    ini = 0.0 if c == 0 else hT[:, o - 1:o]
                    nc.vector.tensor_tensor_scan(hT[:, o:o + SUB], aT[:], bT[:],
                                                 ini, AOT.mult, AOT.add)
                for c in range(4):
                    o = OSUB * c
                    hcT = Tp.tile([128, OSUB], F32R, tag="hcR")
                    nc.gpsimd.tensor_tensor(hcT[:], hT[:, 64 + o:64 + o + OSUB],
                                            CbT[:, 64 + o:64 + o + OSUB], AOT.mult)
                    mm(pY[:, o:o + OSUB], ct["patyg"][:, 128 * g:128 * (g + 1)],
                       hcT[:], start=(g == 0), stop=(g == 15))

            if STAGE <= 3:
                if i == 0:
                    ySB = Sp.tile([128, TH], F32, tag="oSB")
                    for c in range(4):
                        o = OSUB * c
                        nc.scalar.copy(ySB[:, o:o + OSUB], pY[:, o:o + OSUB])
                    nc.sync.dma_start(out=out[0:128, :], in_=ySB[:])
                    nc.sync.dma_start(out=out[128:256, :], in_=CbT[:, 64:])
                continue
            # ---- gating, out_proj, LN1 stats (act set 3: Square/Sqrt) ----
            mf_t = mfin[i // 2]
            t5 = Tp.tile([128, TH], F32, tag="t5c", bufs=1)
            nc.vector.scalar_tensor_tensor(t5[:], xcT[:, 64:64 + TH].bitcast(F32),
                                           ct["dp"][:], pY[:, 0:TH],
                                           AOT.mult, AOT.add)
            t6 = Tp.tile([128, TH], F32R, tag="t6c", bufs=1)
            nc.gpsimd.tensor_tensor(t6[:], t5[:], szT[:, 64:64 + TH], AOT.mult)
            mnT = Sp.tile([64, TH], F32R, tag="mnT")
            for c in range(4):
                o = OSUB * c
                pm = psM.tile([64, OSUB], F32, tag="pmm")
                mm(pm[:], ct["opw"][:], t6[:, o:o + OSUB])
                mSB = Tp.tile([64, OSUB], F32R, tag="mSBc")
                nc.scalar.copy(mSB[:], pm[:])
                ps1 = psM.tile([1, OSUB], F32, tag="pmm")
                mm(ps1[:], ct["onesc"][0:64, :], mSB[:])
                s1 = Tp.tile([1, 512], F32, tag="rA", bufs=1)
                nc.vector.tensor_scalar(out=s1[:], in0=ps1[:],
                                        scalar1=1.0 / D, scalar2=None, op0=AOT.mult)
                sqt = Tp.tile([64, OSUB], F32R, tag="scrR", bufs=1)
                nc.scalar.activation(sqt[:], mSB[:].bitcast(F32), AFT.Square)
                pq1 = psM.tile([1, OSUB], F32, tag="pmm")
                mm(pq1[:], ct["onesc"][0:64, :], sqt[:])
                q1 = Tp.tile([1, 512], F32, tag="rB", bufs=1)
                nc.vector.tensor_scalar(out=q1[:], in0=pq1[:],
                                        scalar1=1.0 / D, scalar2=None, op0=AOT.mult)
                m2b = Tp.tile([1, 512], F32, tag="rC", bufs=1)
                nc.vector.tensor_tensor(m2b[:], s1[:], s1[:], AOT.mult)
                v1 = Tp.tile([1, 512], F32, tag="rD", bufs=1)
                nc.vector.tensor_tensor(v1[:], q1[:], m2b[:], AOT.subtract)
                sd1 = Tp.tile([1, 512], F32, tag="rC", bufs=1)
                nc.scalar.activation(sd1[:], v1[:], AFT.Sqrt, bias=eps_t[:])
                rs1 = Tp.tile([1, 512], F32, tag="rD", bufs=1)
                nc.vector.reciprocal_approx_fast(rs1[:], sd1[:])
                bmn = Tp.tile([64, OSUB], F32, tag="bcA", bufs=1)
                nc.gpsimd.partition_broadcast(bmn[:], s1[:])
                brs = Tp.tile([64, OSUB], F32, tag="bcB", bufs=1)
                nc.gpsimd.partition_broadcast(brs[:], rs1[:])
                tq = Tp.tile([64, OSUB], F32, tag="scr")
                nc.vector.scalar_tensor_tensor(tq[:], mSB[:].bitcast(F32), 1.0,
                                               bmn[:], AOT.mult, AOT.subtract)
                nc.vector.scalar_tensor_tensor(mnT[:, o:o + OSUB], tq[:], 1.0,
                                               brs[:], AOT.mult, AOT.mult)
            # ---- MLP (act set 10: Gelu) + skip ----
            for c in range(4):
                o = OSUB * c
                ph1 = psM.tile([128, OSUB], F32, tag="pmm")
                mm(ph1[:], ct["fc1"][:, 0:128], mnT[:, o:o + OSUB])
                h1 = Tp.tile([128, OSUB], F32R, tag="h1a")
                nc.scalar.activation(h1[:], ph1[:], AFT.Gelu, bias=ct["fc1b"][:, 0:1])
                ph2 = psM.tile([128, OSUB], F32, tag="pmm")
                mm(ph2[:], ct["fc1"][:, 128:256], mnT[:, o:o + OSUB])
                h2 = Tp.tile([128, OSUB], F32R, tag="h1b")
                nc.scalar.activation(h2[:], ph2[:], AFT.Gelu, bias=ct["fc1b"][:, 1:2])
                pf2 = psM.tile([64, OSUB], F32, tag="pmm")
                mm(pf2[:], ct["fc2"][:, 0:64], h1[:],
                   start=True, stop=False)
                mm(pf2[:], ct["fc2"][:, 64:128], h2[:],
                   start=False, stop=False)
                mm(pf2[:], ct["fc2br"][:], ct["onesr"][:],
                   start=False, stop=True)
                nc.vector.scalar_tensor_tensor(mf_t[r0:r0 + 64, o:o + OSUB],
                                               xnh[r0:r0 + 64, PAD + o:PAD + o + OSUB].bitcast(F32),
                                               ct["skips"][r0:r0 + 64, :],
                                               pf2[:], AOT.mult, AOT.add)

        if STAGE == 4:
            for half in range(2):
                nc.sync.dma_start(out=out[128 * half:128 * (half + 1), :],
                                  in_=mfin[half][:].bitcast(F32))
        # ==== 1x1 conv across chunks + BN + SiLU ====
        for half in range(2 if STAGE >= 5 else 0):
            oSB = Sp.tile([128, TH], F32, tag="oSB")
            for c in range(4):
                o = OSUB * c
                pyc = psM.tile([128, OSUB], F32, tag="pmm")
                for t in range(2):
                    mm(pyc[:], ct["wout"][:, t * C + 128 * half:t * C + 128 * (half + 1)],
                       mfin[t][:, o:o + OSUB], start=(t == 0), stop=(t == 1))
                nc.scalar.activation(oSB[:, o:o + OSUB], pyc[:], AFT.Silu,
                                     scale=ct["bnsc"][:, half:half + 1],
                                     bias=ct["bnsh"][:, half:half + 1])
            nc.sync.dma_start(out=out[128 * half:128 * (half + 1), :], in_=oSB[:])

    nc.compile()
    _cache["nc"] = nc
    return nc


def _host_prep(inputs):
    f32 = np.float32

    def a(k):
        return np.asarray(inputs[k], f32)

    g, b_, Win = a("ln_g"), a("ln_b"), a("in_proj_w")
    convw, convb = a("conv_w"), a("conv_b")
    com = {}
    wctap = np.zeros((D, 16 * DI), f32)
    wz = np.zeros((D, 4 * DI), f32)
    ccv = np.zeros((DI, 4), f32)
    cz = np.zeros((DI, 4), f32)
    for i in range(4):
        gi, bi = g[64 * i:64 * (i + 1)], b_[64 * i:64 * (i + 1)]
        wxc = gi[:, None] * Win[:, :DI]
        for j in range(DC):
            wctap[:, (4 * i + j) * DI:(4 * i + j + 1) * DI] = wxc * convw[None, :, j]
        wz[:, i * DI:(i + 1) * DI] = gi[:, None] * Win[:, DI:]
        ccv[:, i] = (bi @ Win[:, :DI]) * convw.sum(1) + convb
        cz[:, i] = bi @ Win[:, DI:]
    com["wctap"], com["wz"] = np.tile(wctap, (2, 1)), np.tile(wz, (2, 1))
    com["ccv"], com["cz"] = ccv, cz
    xpw_raw = a("x_proj_w")
    xpw = np.zeros((DI, 96), f32)
    xpw[:, 0:DTR] = xpw_raw[:, 0:DTR]
    xpw[:, 32:48] = xpw_raw[:, DTR:DTR + DS]
    xpw[:, 64:80] = xpw_raw[:, DTR + DS:]
    com["xpw"] = xpw
    com["dtw"] = a("dt_proj_w")
    com["dtb"] = a("dt_proj_b").reshape(DI, 1)
    com["ndtb"] = -a("dt_proj_b").reshape(DI, 1)
    A = -np.exp(a("A_log"))
    acols = np.zeros((128, 16), f32)
    for p in range(128):
        for gg in range(16):
            acols[p, gg] = A[8 * gg + p // 16, p % 16]
    com["acols"] = acols
    com["dp"] = a("Dparam").reshape(DI, 1)
    com["opw"] = a("out_proj_w")
    g1, b1, fc1w = a("ln1_g"), a("ln1_b"), a("fc1_w")
    com["fc1"] = g1[:, None] * fc1w
    com["fc1b"] = (a("fc1_b") + b1 @ fc1w).reshape(2, 128).T.copy()
    fc2w = a("fc2_w")
    com["fc2"] = np.concatenate([fc2w[0:128, :], fc2w[128:256, :]], axis=1)
    com["fc2br"] = a("fc2_b").reshape(1, D)
    outcw = a("outc_w")
    wout = np.zeros((128, 2 * C), f32)
    for t in range(2):
        for i in (2 * t, 2 * t + 1):
            for d in range(D):
                wout[64 * (i % 2) + d, t * C:(t + 1) * C] = outcw[:, 4 * d + i]
    com["wout"] = wout
    sc = a("bn_g") / np.sqrt(a("bn_v") + EPS)
    com["bnsc"] = sc.reshape(2, 128).T.copy()
    com["bnsh"] = (a("bn_b") - a("bn_m") * sc).reshape(2, 128).T.copy()
    patg = np.zeros((128, 16 * 128), f32)
    patyg = np.zeros((128, 16 * 128), f32)
    for gg in range(16):
        for p in range(128):
            patg[8 * gg + p // 16, 128 * gg + p] = 1.0    # bcast d-row -> (d,s)
            patyg[p, 128 * gg + 8 * gg + p // 16] = 1.0   # sum over s -> d row
    patsbc = np.zeros((128, 256), f32)
    for p in range(128):
        patsbc[32 + p % 16, p] = 1.0          # B bcast lhsT rows 32:48
        patsbc[64 + p % 16, 128 + p] = 1.0    # C bcast lhsT rows 64:80
    com["patg"], com["patyg"], com["patsbc"] = patg, patyg, patsbc
    com["onesr"] = np.ones((1, 512), f32)
    com["onesc"] = np.ones((128, 1), f32)
    com["skips"] = np.full((128, 1), float(np.asarray(inputs["skip_scale"]).reshape(-1)[0]), f32)
    return {k: np.ascontiguousarray(v, f32) for k, v in com.items()}


def kernel(**inputs):
    nc = _build()
    com = _host_prep(inputs)
    x = np.asarray(inputs["x"], np.float32).reshape(B, C, N)
    in_maps = []
    for k in range(8):
        b, half = k // 2, k % 2
        if half == 0:
            xs = np.concatenate([np.zeros((C, PAD), np.float32), x[b, :, :TH]], axis=1)
        else:
            xs = x[b, :, TH - PAD:N]
        m = {"xs": np.ascontiguousarray(xs)}
        m.update(com)
        in_maps.append(m)
    res = run_bass_kernel_spmd(nc, in_maps, core_ids=list(range(8)))
    outp = np.zeros((B, C, N), np.float32)
    for k in range(8):
        b, half = k // 2, k % 2
        outp[b, :, half * TH:(half + 1) * TH] = res.results[k]["out"]
    return outp.reshape(B, C, H, W)


# revision 16
# speedup vs baseline: 1.4890x; 1.1297x over previous
"""Trainium2 Bass kernel for nn_CSI_75453985457421 (LN + chunked Mamba + MLP + 1x1conv + BN + SiLU).

Sharding: 8 cores = (batch b 0..3) x (time-half 0..1). Each core gets
x[b, :, half*2048-67 : half*2048+2048] (zero-padded before the sequence start)
and computes its 2048 output positions independently: 67 warmup columns
(3 causal-conv pad + 64 scan warmup; state decay <= exp(-0.68*64) << fp32 eps).

Device layout: time on the free axis. The selective scan runs with partitions
= (d_local, s): 16 groups of 8 d-channels x 16 states via the hardware
tensor_tensor_scan (DVE). dt/dtu/B/C broadcasts and the final sum over s are
TensorE pattern matmuls in float32r (1 cycle/row vs 4 for fp32); exp(A*dt) is
ScalarE with a per-partition scale. LN gamma/beta, the depthwise conv, the
channel interleave and BatchNorm are folded into weights on the host.
Stat broadcasts run on the idle GpSimd/Pool engine (partition_broadcast).
"""
import os
import sys

sys.path.insert(0, "/opt/trn_rl_repo")
STAGE = int(os.environ.get("KSTAGE", "9"))
import numpy as np
import concourse.bass as bass
import concourse.bacc as bacc
import concourse.tile as tile
from concourse import mybir
from concourse.bass_utils import run_bass_kernel_spmd

F32 = mybir.dt.float32
F32R = mybir.dt.float32r
AOT = mybir.AluOpType
AFT = mybir.ActivationFunctionType

B, C, H, W = 4, 256, 64, 64
N = H * W
D, DI, DS, DC, DTR, MH = 64, 128, 16, 4, 4, 256
EPS = 1e-5
PAD = 67
TH = 2048
TEXT = PAD + TH          # 2115
SCT = TEXT - 3           # 2112 = 4*528
SUB = 528
OSUB = 512

import functools
import concourse.hw_specs as _hw_specs
import concourse.bacc as _bacc_mod

_real_get_tables = _hw_specs.get_activation_tables

@functools.cache
def _patched_get_tables(arch):
    out = {}
    for name, s in _real_get_tables(arch).items():
        s = set(s)
        if name in ("exp_and_others", "exp_and_friends"):
            s.discard(AFT.Exp)
        if name == "natural_log":
            s.discard(AFT.Ln)
        out[name] = s
    return out

_hw_specs.get_activation_tables = _patched_get_tables
_bacc_mod.get_activation_tables = _patched_get_tables

_cache = {}

# name -> (shape, is_matmul_operand)
_IN_SHAPES = dict(
    xs=((C, TEXT), True), wctap=((128, 16 * DI), True), wz=((128, 4 * DI), True),
    ccv=((DI, 4), False), cz=((DI, 4), False),
    xpw=((DI, 96), True), dtw=((DTR, DI), True), dtb=((DI, 1), False),
    acols=((128, 16), False), dp=((DI, 1), False), ndtb=((DI, 1), False),
    opw=((DI, D), True), fc1=((D, MH), True), fc1b=((128, 2), False),
    fc2=((128, 2 * D), True), fc2br=((1, D), True),
    wout=((128, 2 * C), True), bnsc=((128, 2), False), bnsh=((128, 2), False),
    patg=((128, 16 * 128), True), patyg=((128, 16 * 128), True),
    patsbc=((128, 256), True), onesr=((1, 512), True), onesc=((128, 1), True),
    skips=((128, 1), False),
)


def _build():
    if "nc" in _cache:
        return _cache["nc"]
    nc = bacc.Bacc("TRN2", target_bir_lowering=False, debug=False, num_devices=8)
    dram = {k: nc.dram_tensor(k, list(s), F32, kind="ExternalInput").ap()
            for k, (s, _) in _IN_SHAPES.items()}
    out = nc.dram_tensor("out", [C, TH], F32, kind="ExternalOutput").ap()

    with tile.TileContext(nc) as tc, \
            tc.tile_pool(name="const", bufs=1) as Kp, \
            tc.tile_pool(name="big", bufs=1) as Bp, \
            tc.tile_pool(name="seq", bufs=1) as Sp, \
            tc.tile_pool(name="tmp", bufs=2) as Tp, \
            tc.tile_pool(name="scan", bufs=2) as Cp, \
            tc.tile_pool(name="psA", bufs=1, space="PSUM") as psA, \
            tc.tile_pool(name="psM", bufs=1, space="PSUM") as psM, \
            tc.tile_pool(name="psY", bufs=1, space="PSUM") as psY:

        def mm(out_ap, lhsT, rhs, start=True, stop=True):
            n = out_ap.shape[-1]
            if n <= 512:
                nc.tensor.matmul(out_ap, lhsT, rhs, start=start, stop=stop)
                return
            o = 0
            while o < n:
                w_ = min(512, n - o)
                nc.tensor.matmul(out_ap[..., o:o + w_], lhsT, rhs[..., o:o + w_],
                                 start=start, stop=stop)
                o += w_

        ct = {}
        for k, (shp, is_r) in _IN_SHAPES.items():
            if k == "xs":
                continue
            dt_ = F32R if is_r else F32
            ct[k] = Kp.tile(list(shp), dt_, tag=k, name=f"ct_{k}")
            src = dram[k][:].bitcast(F32R) if is_r else dram[k][:]
            nc.sync.dma_start(out=ct[k][:], in_=src)
        eps_t = Kp.tile([1, 1], F32, tag="eps")
        nc.vector.memset(eps_t[:], EPS)

        xh = [Bp.tile([128, TEXT], F32R, tag=f"xh{h}", name=f"xh{h}") for h in range(2)]
        for h in range(2):
            nc.sync.dma_start(out=xh[h][:], in_=dram["xs"][128 * h:128 * (h + 1), :].bitcast(F32R))

        # ---- LayerNorm over C: fused per-subtile stats + apply ----
        # last subtile overlaps col 2047 so every width stays even (fp32r
        # matmul requires an even moving width); re-normalizing an already
        # normalized column is a ~eps no-op.
        nsub = [(0, 512), (512, 512), (1024, 512), (1536, 512), (TEXT - 68, 68)]
        for (o, w_) in nsub:
            pse = psM.tile([1, 512], F32, tag="pmm")
            for h in range(2):
                mm(pse[:, :w_], ct["onesc"][:], xh[h][:, o:o + w_],
                   start=(h == 0), stop=(h == 1))
            mean = Tp.tile([1, 512], F32, tag="rA", bufs=1)
            nc.vector.tensor_scalar(out=mean[:, :w_], in0=pse[:, :w_],
                                    scalar1=1.0 / C, scalar2=None, op0=AOT.mult)
            psq = psM.tile([1, 512], F32, tag="pmm")
            for h in range(2):
                sqt = Tp.tile([128, 512], F32R, tag="scrR", bufs=1)
                nc.scalar.activation(sqt[:, :w_], xh[h][:, o:o + w_].bitcast(F32),
                                     AFT.Square)
                mm(psq[:, :w_], ct["onesc"][:], sqt[:, :w_],
                   start=(h == 0), stop=(h == 1))
            sqm = Tp.tile([1, 512], F32, tag="rB", bufs=1)
            nc.vector.tensor_scalar(out=sqm[:, :w_], in0=psq[:, :w_],
                                    scalar1=1.0 / C, scalar2=None, op0=AOT.mult)
            m2 = Tp.tile([1, 512], F32, tag="rC", bufs=1)
            nc.vector.tensor_tensor(m2[:, :w_], mean[:, :w_], mean[:, :w_], AOT.mult)
            var = Tp.tile([1, 512], F32, tag="rD", bufs=1)
            nc.vector.tensor_tensor(var[:, :w_], sqm[:, :w_], m2[:, :w_], AOT.subtract)
            sd = Tp.tile([1, 512], F32, tag="rC", bufs=1)
            nc.scalar.activation(sd[:, :w_], var[:, :w_], AFT.Sqrt, bias=eps_t[:])
            rstd = Tp.tile([1, 512], F32, tag="rD", bufs=1)
            nc.vector.reciprocal_approx_fast(rstd[:, :w_], sd[:, :w_])
            bmean = Tp.tile([128, 512], F32, tag="bcA", bufs=1)
            nc.gpsimd.partition_broadcast(bmean[:, :w_], mean[:, :w_])
            brstd = Tp.tile([128, 512], F32, tag="bcB", bufs=1)
            nc.gpsimd.partition_broadcast(brstd[:, :w_], rstd[:, :w_])
            for h in range(2):
                tmp = Tp.tile([128, 512], F32, tag="scr")
                nc.vector.scalar_tensor_tensor(tmp[:, :w_], xh[h][:, o:o + w_].bitcast(F32),
                                               1.0, bmean[:, :w_], AOT.mult, AOT.subtract)
                nc.vector.scalar_tensor_tensor(xh[h][:, o:o + w_], tmp[:, :w_], 1.0,
                                               brstd[:, :w_], AOT.mult, AOT.mult)

        mfin = [Bp.tile([128, TH], F32R, tag=f"mfin{h}", name=f"mfin{h}") for h in range(2)]
        if STAGE <= 1:
            for half in range(2):
                nc.sync.dma_start(out=out[128 * half:128 * (half + 1), :],
                                  in_=xh[half][:, PAD:].bitcast(F32))
        nseq = 0 if STAGE <= 1 else 4
        # ==== per sequence (channel chunk) ====
        # Emission is batched by activation-table set to minimize 1.3us
        # LoadActFuncSet switches: pass1 Silu/Copy (set 18), pass2+scan
        # Exp/Ln (set 6), MLP stats Square/Sqrt (set 3), Gelu (set 10).
        for i in range(nseq):
            xnh = xh[i // 2]
            r0 = 64 * (i % 2)
            xcT = Sp.tile([128, SCT], F32R, tag="xcT")
            szT = Sp.tile([128, SCT], mybir.dt.bfloat16, tag="szT")
            dtT = Sp.tile([128, SCT], F32R, tag="dtT")
            dtuT = Sp.tile([128, SCT], F32R, tag="dtuT")
            BbT = Sp.tile([128, SCT], mybir.dt.bfloat16, tag="BbT")
            CbT = Sp.tile([128, SCT], mybir.dt.bfloat16, tag="CbT")
            xdblT = Sp.tile([96, SCT], F32R, tag="xdblT")

            # -- pass 1: projections + Silu (act set 18) + PSUM copies --
            for c in range(4):
                o = SUB * c
                pxt = psA.tile([128, SUB], F32, tag="pbc")
                for j in range(DC):
                    mm(pxt[:], ct["wctap"][r0:r0 + 64, (4 * i + j) * DI:(4 * i + j + 1) * DI],
                       xnh[r0:r0 + 64, o + j:o + j + SUB],
                       start=(j == 0), stop=(j == DC - 1))
                nc.scalar.activation(xcT[:, o:o + SUB], pxt[:], AFT.Silu,
                                     bias=ct["ccv"][:, i:i + 1])
                pz = psM.tile([128, SUB], F32, tag="pmm")
                mm(pz[:], ct["wz"][r0:r0 + 64, i * DI:(i + 1) * DI],
                   xnh[r0:r0 + 64, o + 3:o + 3 + SUB])
                nc.scalar.activation(szT[:, o:o + SUB], pz[:], AFT.Silu,
                                     bias=ct["cz"][:, i:i + 1])
                pxd = psA.tile([96, SUB], F32, tag="pbc")
                mm(pxd[:], ct["xpw"][:], xcT[:, o:o + SUB])
                nc.scalar.copy(xdblT[:, o:o + SUB], pxd[:])
                pbb = psA.tile([128, SUB], F32, tag="pbc")
                mm(pbb[:], ct["patsbc"][32:48, 0:128], xdblT[32:48, o:o + SUB])
                nc.scalar.copy(BbT[:, o:o + SUB], pbb[:])
                pcb = psM.tile([128, SUB], F32, tag="pmm")
                mm(pcb[:], ct["patsbc"][64:80, 128:256], xdblT[64:80, o:o + SUB])
                nc.scalar.copy(CbT[:, o:o + SUB], pcb[:])

            # -- pass 2: dt softplus (act set 6: Exp/Ln) + dtu on Pool --
            for c in range(4):
                o = SUB * c
                pdt = psM.tile([128, SUB], F32, tag="pmm")
                mm(pdt[:], ct["dtw"][:], xdblT[0:4, o:o + SUB])
                # softplus(x) = x + ln(1 + exp(-x)); x = pdt + dtb
                eneg = Tp.tile([128, SUB], F32, tag="spe", bufs=1)
                nc.scalar.activation(eneg[:], pdt[:], AFT.Exp, scale=-1.0,
                                     bias=ct["ndtb"][:])
                lnv = Tp.tile([128, SUB], F32, tag="spl", bufs=1)
                nc.scalar.activation(lnv[:], eneg[:], AFT.Ln, bias=1.0)
                nc.vector.scalar_tensor_tensor(dtT[:, o:o + SUB], pdt[:],
                                               ct["dtb"][:], lnv[:],
                                               AOT.add, AOT.add)
                nc.gpsimd.tensor_tensor(dtuT[:, o:o + SUB], dtT[:, o:o + SUB].bitcast(F32),
                                        xcT[:, o:o + SUB].bitcast(F32), AOT.mult)

            # ---- selective scan over 16 (d-group) x 16 (state) partitions ----
            if STAGE <= 2:
                if i == 0:
                    nc.sync.dma_start(out=out[0:128, :], in_=dtT[:, 64:].bitcast(F32))
                    nc.sync.dma_start(out=out[128:256, :], in_=BbT[:, 64:])
                continue
            pY = psY.tile([128, TH], F32, tag="py")
            for g in range(16):
                hT = Cp.tile([128, SCT], F32, tag="hT", bufs=2)
                for c in range(4):
                    o = SUB * c
                    aT = Cp.tile([128, SUB], F32, tag="aT")
                    bT = Cp.tile([128, SUB], F32, tag="bT")
                    pda = psA.tile([128, SUB], F32, tag="pbc")
                    mm(pda[:], ct["patg"][:, 128 * g:128 * (g + 1)], dtT[:, o:o + SUB])
                    nc.scalar.activation(aT[:], pda[:], AFT.Exp,
                                         scale=ct["acols"][:, g:g + 1])
                    pdu = psM.tile([128, SUB], F32, tag="pmm")
                    mm(pdu[:], ct["patg"][:, 128 * g:128 * (g + 1)], dtuT[:, o:o + SUB])
                    nc.vector.scalar_tensor_tensor(bT[:], pdu[:], 1.0,
                                                   BbT[:, o:o + SUB],
                                                   AOT.mult, AOT.mult)
                    ini = 0.0 if c == 0 else hT[:, o - 1:o]
                    nc.vector.tensor_tensor_scan(hT[:, o:o + SUB], aT[:], bT[:],
                                                 ini, AOT.mult, AOT.add)
                for c in range(4):
                    o = OSUB * c
                    hcT = Tp.tile([128, OSUB], F32R, tag="hcR")
                    nc.gpsimd.tensor_tensor(hcT[:], hT[:, 64 + o:64 + o + OSUB],
                                            CbT[:, 64 + o:64 + o + OSUB], AOT.mult)
                    mm(pY[:, o:o + OSUB], ct["patyg"][:, 128 * g:128 * (g + 1)],
                       hcT[:], start=(g == 0), stop=(g == 15))

            if STAGE <= 3:
                if i == 0:
                    ySB = Sp.tile([128, TH], F32, tag="oSB")
                    for c in range(4):
                        o = OSUB * c
                        nc.scalar.copy(ySB[:, o:o + OSUB], pY[:, o:o + OSUB])
                    nc.sync.dma_start(out=out[0:128, :], in_=ySB[:])
                    nc.sync.dma_start(out=out[128:256, :], in_=CbT[:, 64:])
                continue
            # ---- gating, out_proj, LN1 stats (act set 3: Square/Sqrt) ----
            mf_t = mfin[i // 2]
            t5 = Tp.tile([128, TH], F32, tag="t5c", bufs=1)
            nc.vector.scalar_tensor_tensor(t5[:], xcT[:, 64:64 + TH].bitcast(F32),
                                           ct["dp"][:], pY[:, 0:TH],
                                           AOT.mult, AOT.add)
            t6 = Tp.tile([128, TH], F32R, tag="t6c", bufs=1)
            nc.gpsimd.tensor_tensor(t6[:], t5[:], szT[:, 64:64 + TH], AOT.mult)
            mnT = Sp.tile([64, TH], F32R, tag="mnT")
            for c in range(4):
                o = OSUB * c
                pm = psM.tile([64, OSUB], F32, tag="pmm")
                mm(pm[:], ct["opw"][:], t6[:, o:o + OSUB])
                mSB = Tp.tile([64, OSUB], F32R, tag="mSBc")
                nc.scalar.copy(mSB[:], pm[:])
                ps1 = psM.tile([1, OSUB], F32, tag="pmm")
                mm(ps1[:], ct["onesc"][0:64, :], mSB[:])
                s1 = Tp.tile([1, 512], F32, tag="rA", bufs=1)
                nc.vector.tensor_scalar(out=s1[:], in0=ps1[:],
                                        scalar1=1.0 / D, scalar2=None, op0=AOT.mult)
                sqt = Tp.tile([64, OSUB], F32R, tag="scrR", bufs=1)
                nc.scalar.activation(sqt[:], mSB[:].bitcast(F32), AFT.Square)
                pq1 = psM.tile([1, OSUB], F32, tag="pmm")
                mm(pq1[:], ct["onesc"][0:64, :], sqt[:])
                q1 = Tp.tile([1, 512], F32, tag="rB", bufs=1)
                nc.vector.tensor_scalar(out=q1[:], in0=pq1[:],
                                        scalar1=1.0 / D, scalar2=None, op0=AOT.mult)
                m2b = Tp.tile([1, 512], F32, tag="rC", bufs=1)
                nc.vector.tensor_tensor(m2b[:], s1[:], s1[:], AOT.mult)
                v1 = Tp.tile([1, 512], F32, tag="rD", bufs=1)
                nc.vector.tensor_tensor(v1[:], q1[:], m2b[:], AOT.subtract)
                sd1 = Tp.tile([1, 512], F32, tag="rC", bufs=1)
                nc.scalar.activation(sd1[:], v1[:], AFT.Sqrt, bias=eps_t[:])
                rs1 = Tp.tile([1, 512], F32, tag="rD", bufs=1)
                nc.vector.reciprocal_approx_fast(rs1[:], sd1[:])
                bmn = Tp.tile([64, OSUB], F32, tag="bcA", bufs=1)
                nc.gpsimd.partition_broadcast(bmn[:], s1[:])
                brs = Tp.tile([64, OSUB], F32, tag="bcB", bufs=1)
                nc.gpsimd.partition_broadcast(brs[:], rs1[:])
                tq = Tp.tile([64, OSUB], F32, tag="scr")
                nc.vector.scalar_tensor_tensor(tq[:], mSB[:].bitcast(F32), 1.0,
                                               bmn[:], AOT.mult, AOT.subtract)
                nc.vector.scalar_tensor_tensor(mnT[:, o:o + OSUB], tq[:], 1.0,
                                               brs[:], AOT.mult, AOT.mult)
            # ---- MLP (act set 10: Gelu) + skip ----
            for c in range(4):
                o = OSUB * c
                ph1 = psM.tile([128, OSUB], F32, tag="pmm")
                mm(ph1[:], ct["fc1"][:, 0:128], mnT[:, o:o + OSUB])
                h1 = Tp.tile([128, OSUB], F32R, tag="h1a")
                nc.scalar.activation(h1[:], ph1[:], AFT.Gelu, bias=ct["fc1b"][:, 0:1])
                ph2 = psM.tile([128, OSUB], F32, tag="pmm")
                mm(ph2[:], ct["fc1"][:, 128:256], mnT[:, o:o + OSUB])
                h2 = Tp.tile([128, OSUB], F32R, tag="h1b")
                nc.scalar.activation(h2[:], ph2[:], AFT.Gelu, bias=ct["fc1b"][:, 1:2])
                pf2 = psM.tile([64, OSUB], F32, tag="pmm")
                mm(pf2[:], ct["fc2"][:, 0:64], h1[:],
                   start=True, stop=False)
                mm(pf2[:], ct["fc2"][:, 64:128], h2[:],
                   start=False, stop=False)
                mm(pf2[:], ct["fc2br"][:], ct["onesr"][:],
                   start=False, stop=True)
                nc.vector.scalar_tensor_tensor(mf_t[r0:r0 + 64, o:o + OSUB],
                                               xnh[r0:r0 + 64, PAD + o:PAD + o + OSUB].bitcast(F32),
                                               ct["skips"][r0:r0 + 64, :],
                                               pf2[:], AOT.mult, AOT.add)

        if STAGE == 4:
            for half in range(2):
                nc.sync.dma_start(out=out[128 * half:128 * (half + 1), :],
                                  in_=mfin[half][:].bitcast(F32))
        # ==== 1x1 conv across chunks + BN + SiLU ====
        for half in range(2 if STAGE >= 5 else 0):
            oSB = Sp.tile([128, TH], F32, tag="oSB")
            for c in range(4):
                o = OSUB * c
                pyc = psM.tile([128, OSUB], F32, tag="pmm")
                for t in range(2):
                    mm(pyc[:], ct["wout"][:, t * C + 128 * half:t * C + 128 * (half + 1)],
                       mfin[t][:, o:o + OSUB], start=(t == 0), stop=(t == 1))
                nc.scalar.activation(oSB[:, o:o + OSUB], pyc[:], AFT.Silu,
                                     scale=ct["bnsc"][:, half:half + 1],
                                     bias=ct["bnsh"][:, half:half + 1])
            nc.sync.dma_start(out=out[128 * half:128 * (half + 1), :], in_=oSB[:])

    nc.compile()
    _cache["nc"] = nc
    return nc


def _host_prep(inputs):
    f32 = np.float32

    def a(k):
        return np.asarray(inputs[k], f32)

    g, b_, Win = a("ln_g"), a("ln_b"), a("in_proj_w")
    convw, convb = a("conv_w"), a("conv_b")
    com = {}
    wctap = np.zeros((D, 16 * DI), f32)
    wz = np.zeros((D, 4 * DI), f32)
    ccv = np.zeros((DI, 4), f32)
    cz = np.zeros((DI, 4), f32)
    for i in range(4):
        gi, bi = g[64 * i:64 * (i + 1)], b_[64 * i:64 * (i + 1)]
        wxc = gi[:, None] * Win[:, :DI]
        for j in range(DC):
            wctap[:, (4 * i + j) * DI:(4 * i + j + 1) * DI] = wxc * convw[None, :, j]
        wz[:, i * DI:(i + 1) * DI] = gi[:, None] * Win[:, DI:]
        ccv[:, i] = (bi @ Win[:, :DI]) * convw.sum(1) + convb
        cz[:, i] = bi @ Win[:, DI:]
    com["wctap"], com["wz"] = np.tile(wctap, (2, 1)), np.tile(wz, (2, 1))
    com["ccv"], com["cz"] = ccv, cz
    xpw_raw = a("x_proj_w")
    xpw = np.zeros((DI, 96), f32)
    xpw[:, 0:DTR] = xpw_raw[:, 0:DTR]
    xpw[:, 32:48] = xpw_raw[:, DTR:DTR + DS]
    xpw[:, 64:80] = xpw_raw[:, DTR + DS:]
    com["xpw"] = xpw
    com["dtw"] = a("dt_proj_w")
    com["dtb"] = a("dt_proj_b").reshape(DI, 1)
    com["ndtb"] = -a("dt_proj_b").reshape(DI, 1)
    A = -np.exp(a("A_log"))
    acols = np.zeros((128, 16), f32)
    for p in range(128):
        for gg in range(16):
            acols[p, gg] = A[8 * gg + p // 16, p % 16]
    com["acols"] = acols
    com["dp"] = a("Dparam").reshape(DI, 1)
    com["opw"] = a("out_proj_w")
    g1, b1, fc1w = a("ln1_g"), a("ln1_b"), a("fc1_w")
    com["fc1"] = g1[:, None] * fc1w
    com["fc1b"] = (a("fc1_b") + b1 @ fc1w).reshape(2, 128).T.copy()
    fc2w = a("fc2_w")
    com["fc2"] = np.concatenate([fc2w[0:128, :], fc2w[128:256, :]], axis=1)
    com["fc2br"] = a("fc2_b").reshape(1, D)
    outcw = a("outc_w")
    wout = np.zeros((128, 2 * C), f32)
    for t in range(2):
        for i in (2 * t, 2 * t + 1):
            for d in range(D):
                wout[64 * (i % 2) + d, t * C:(t + 1) * C] = outcw[:, 4 * d + i]
    com["wout"] = wout
    sc = a("bn_g") / np.sqrt(a("bn_v") + EPS)
    com["bnsc"] = sc.reshape(2, 128).T.copy()
    com["bnsh"] = (a("bn_b") - a("bn_m") * sc).reshape(2, 128).T.copy()
    patg = np.zeros((128, 16 * 128), f32)
    patyg = np.zeros((128, 16 * 128), f32)
    for gg in range(16):
        for p in range(128):
            patg[8 * gg + p // 16, 128 * gg + p] = 1.0    # bcast d-row -> (d,s)
            patyg[p, 128 * gg + 8 * gg + p // 16] = 1.0   # sum over s -> d row
    patsbc = np.zeros((128, 256), f32)
    for p in range(128):
        patsbc[32 + p % 16, p] = 1.0          # B bcast lhsT rows 32:48
        patsbc[64 + p % 16, 128 + p] = 1.0    # C bcast lhsT rows 64:80
    com["patg"], com["patyg"], com["patsbc"] = patg, patyg, patsbc
    com["onesr"] = np.ones((1, 512), f32)
    com["onesc"] = np.ones((128, 1), f32)
    com["skips"] = np.full((128, 1), float(np.asarray(inputs["skip_scale"]).reshape(-1)[0]), f32)
    return {k: np.ascontiguousarray(v, f32) for k, v in com.items()}


def kernel(**inputs):
    nc = _build()
    com = _host_prep(inputs)
    x = np.asarray(inputs["x"], np.float32).reshape(B, C, N)
    in_maps = []
    for k in range(8):
        b, half = k // 2, k % 2
        if half == 0:
            xs = np.concatenate([np.zeros((C, PAD), np.float32), x[b, :, :TH]], axis=1)
        else:
            xs = x[b, :, TH - PAD:N]
        m = {"xs": np.ascontiguousarray(xs)}
        m.update(com)
        in_maps.append(m)
    res = run_bass_kernel_spmd(nc, in_maps, core_ids=list(range(8)))
    outp = np.zeros((B, C, N), np.float32)
    for k in range(8):
        b, half = k // 2, k % 2
        outp[b, :, half * TH:(half + 1) * TH] = res.results[k]["out"]
    return outp.reshape(B, C, H, W)


# revision 17
# speedup vs baseline: 1.7434x; 1.1708x over previous
"""Trainium2 Bass kernel for nn_CSI_75453985457421 (LN + chunked Mamba + MLP + 1x1conv + BN + SiLU).

Sharding: 8 cores = (batch b 0..3) x (time-half 0..1). Each core gets
x[b, :, half*2048-67 : half*2048+2048] (zero-padded before the sequence start)
and computes its 2048 output positions independently: 67 warmup columns
(3 causal-conv pad + 64 scan warmup; state decay <= exp(-0.68*64) << fp32 eps).

Device layout: time on the free axis. The selective scan runs with partitions
= (d_local, s): 16 groups of 8 d-channels x 16 states via the hardware
tensor_tensor_scan (DVE). dt/dtu/B/C broadcasts and the final sum over s are
TensorE pattern matmuls in float32r (1 cycle/row vs 4 for fp32); exp(A*dt) is
ScalarE with a per-partition scale. LN gamma/beta, the depthwise conv, the
channel interleave and BatchNorm are folded into weights on the host.
Stat broadcasts run on the idle GpSimd/Pool engine (partition_broadcast).
"""
import os
import sys

sys.path.insert(0, "/opt/trn_rl_repo")
STAGE = int(os.environ.get("KSTAGE", "9"))
import numpy as np
import concourse.bass as bass
import concourse.bacc as bacc
import concourse.tile as tile
from concourse import mybir
from concourse.bass_utils import run_bass_kernel_spmd

F32 = mybir.dt.float32
F32R = mybir.dt.float32r
AOT = mybir.AluOpType
AFT = mybir.ActivationFunctionType

B, C, H, W = 4, 256, 64, 64
N = H * W
D, DI, DS, DC, DTR, MH = 64, 128, 16, 4, 4, 256
EPS = 1e-5
PAD = 67
TH = 2048
TEXT = PAD + TH          # 2115
SCT = TEXT - 3           # 2112 = 4*528
SUB = 528
OSUB = 512

import functools
import concourse.hw_specs as _hw_specs
import concourse.bacc as _bacc_mod

_real_get_tables = _hw_specs.get_activation_tables

@functools.cache
def _patched_get_tables(arch):
    out = {}
    for name, s in _real_get_tables(arch).items():
        s = set(s)
        if name in ("exp_and_others", "exp_and_friends"):
            s.discard(AFT.Exp)
        if name == "natural_log":
            s.discard(AFT.Ln)
        out[name] = s
    return out

_hw_specs.get_activation_tables = _patched_get_tables
_bacc_mod.get_activation_tables = _patched_get_tables

_cache = {}

# name -> (shape, is_matmul_operand)
_IN_SHAPES = dict(
    xs=((C, TEXT), True), wctap=((128, 16 * DI), True), wz=((128, 4 * DI), True),
    ccv=((DI, 4), False), cz=((DI, 4), False),
    xpw=((DI, 96), True), dtw=((DTR, DI), True), dtb=((DI, 1), False),
    acols=((128, 16), False), dp=((DI, 1), False), ndtb=((DI, 1), False),
    opw=((DI, D), True), fc1=((D, MH), True), fc1b=((128, 2), False),
    fc2=((128, 2 * D), True), fc2br=((1, D), True),
    wout=((128, 2 * C), True), bnsc=((128, 2), False), bnsh=((128, 2), False),
    patg=((128, 16 * 128), True), patyg=((128, 16 * 128), True),
    patsbc=((128, 256), True), onesr=((1, 512), True), onesc=((128, 1), True),
    skips=((128, 1), False),
)


def _build():
    if "nc" in _cache:
        return _cache["nc"]
    nc = bacc.Bacc("TRN2", target_bir_lowering=False, debug=False, num_devices=8)
    dram = {k: nc.dram_tensor(k, list(s), F32, kind="ExternalInput").ap()
            for k, (s, _) in _IN_SHAPES.items()}
    out = nc.dram_tensor("out", [C, TH], F32, kind="ExternalOutput").ap()

    with tile.TileContext(nc) as tc, \
            tc.tile_pool(name="const", bufs=1) as Kp, \
            tc.tile_pool(name="big", bufs=1) as Bp, \
            tc.tile_pool(name="seq", bufs=1) as Sp, \
            tc.tile_pool(name="tmp", bufs=2) as Tp, \
            tc.tile_pool(name="scan", bufs=2) as Cp, \
            tc.tile_pool(name="psA", bufs=2, space="PSUM") as psA, \
            tc.tile_pool(name="psM", bufs=2, space="PSUM") as psM, \
            tc.tile_pool(name="psY", bufs=1, space="PSUM") as psY:

        def mm(out_ap, lhsT, rhs, start=True, stop=True):
            n = out_ap.shape[-1]
            if n <= 512:
                nc.tensor.matmul(out_ap, lhsT, rhs, start=start, stop=stop)
                return
            o = 0
            while o < n:
                w_ = min(512, n - o)
                nc.tensor.matmul(out_ap[..., o:o + w_], lhsT, rhs[..., o:o + w_],
                                 start=start, stop=stop)
                o += w_

        ct = {}
        for k, (shp, is_r) in _IN_SHAPES.items():
            if k == "xs":
                continue
            dt_ = F32R if is_r else F32
            ct[k] = Kp.tile(list(shp), dt_, tag=k, name=f"ct_{k}")
            src = dram[k][:].bitcast(F32R) if is_r else dram[k][:]
            nc.sync.dma_start(out=ct[k][:], in_=src)
        eps_t = Kp.tile([1, 1], F32, tag="eps")
        nc.vector.memset(eps_t[:], EPS)

        xh = [Bp.tile([128, TEXT], F32R, tag=f"xh{h}", name=f"xh{h}") for h in range(2)]
        for h in range(2):
            nc.sync.dma_start(out=xh[h][:], in_=dram["xs"][128 * h:128 * (h + 1), :].bitcast(F32R))

        # ---- LayerNorm over C: fused per-subtile stats + apply ----
        # last subtile overlaps col 2047 so every width stays even (fp32r
        # matmul requires an even moving width); re-normalizing an already
        # normalized column is a ~eps no-op.
        nsub = [(0, 512), (512, 512), (1024, 512), (1536, 512), (TEXT - 68, 68)]
        for (o, w_) in nsub:
            pse = psM.tile([1, 512], F32, tag="pmm")
            for h in range(2):
                mm(pse[:, :w_], ct["onesc"][:], xh[h][:, o:o + w_],
                   start=(h == 0), stop=(h == 1))
            mean = Tp.tile([1, 512], F32, tag="rA", bufs=1)
            nc.vector.tensor_scalar(out=mean[:, :w_], in0=pse[:, :w_],
                                    scalar1=1.0 / C, scalar2=None, op0=AOT.mult)
            psq = psM.tile([1, 512], F32, tag="pmm")
            for h in range(2):
                sqt = Tp.tile([128, 512], F32R, tag="scrR", bufs=1)
                nc.scalar.activation(sqt[:, :w_], xh[h][:, o:o + w_].bitcast(F32),
                                     AFT.Square)
                mm(psq[:, :w_], ct["onesc"][:], sqt[:, :w_],
                   start=(h == 0), stop=(h == 1))
            sqm = Tp.tile([1, 512], F32, tag="rB", bufs=1)
            nc.vector.tensor_scalar(out=sqm[:, :w_], in0=psq[:, :w_],
                                    scalar1=1.0 / C, scalar2=None, op0=AOT.mult)
            m2 = Tp.tile([1, 512], F32, tag="rC", bufs=1)
            nc.vector.tensor_tensor(m2[:, :w_], mean[:, :w_], mean[:, :w_], AOT.mult)
            var = Tp.tile([1, 512], F32, tag="rD", bufs=1)
            nc.vector.tensor_tensor(var[:, :w_], sqm[:, :w_], m2[:, :w_], AOT.subtract)
            sd = Tp.tile([1, 512], F32, tag="rC", bufs=1)
            nc.scalar.activation(sd[:, :w_], var[:, :w_], AFT.Sqrt, bias=eps_t[:])
            rstd = Tp.tile([1, 512], F32, tag="rD", bufs=1)
            nc.vector.reciprocal_approx_fast(rstd[:, :w_], sd[:, :w_])
            bmean = Tp.tile([128, 512], F32, tag="bcA", bufs=1)
            nc.gpsimd.partition_broadcast(bmean[:, :w_], mean[:, :w_])
            brstd = Tp.tile([128, 512], F32, tag="bcB", bufs=1)
            nc.gpsimd.partition_broadcast(brstd[:, :w_], rstd[:, :w_])
            for h in range(2):
                tmp = Tp.tile([128, 512], F32, tag="scr")
                nc.vector.scalar_tensor_tensor(tmp[:, :w_], xh[h][:, o:o + w_].bitcast(F32),
                                               1.0, bmean[:, :w_], AOT.mult, AOT.subtract)
                nc.vector.scalar_tensor_tensor(xh[h][:, o:o + w_], tmp[:, :w_], 1.0,
                                               brstd[:, :w_], AOT.mult, AOT.mult)

        mfin = [Bp.tile([128, TH], F32R, tag=f"mfin{h}", name=f"mfin{h}") for h in range(2)]
        if STAGE <= 1:
            for half in range(2):
                nc.sync.dma_start(out=out[128 * half:128 * (half + 1), :],
                                  in_=xh[half][:, PAD:].bitcast(F32))
        CH = [(0, 64), (64, 512), (576, 512), (1088, 512), (1600, 512)]
        nseq = 0 if STAGE <= 1 else 4
        # ==== per sequence (channel chunk) ====
        # Emission is batched by activation-table set to minimize 1.3us
        # LoadActFuncSet switches: pass1 Silu/Copy (set 18), pass2+scan
        # Exp/Ln (set 6), MLP stats Square/Sqrt (set 3), Gelu (set 10).
        for i in range(nseq):
            xnh = xh[i // 2]
            r0 = 64 * (i % 2)
            xcT = Sp.tile([128, SCT], F32R, tag="xcT")
            szT = Sp.tile([128, SCT], mybir.dt.bfloat16, tag="szT")
            dtT = Sp.tile([128, SCT], F32R, tag="dtT")
            dtuT = Sp.tile([128, SCT], F32R, tag="dtuT")
            BbT = Sp.tile([128, SCT], mybir.dt.bfloat16, tag="BbT")
            CbT = Sp.tile([128, SCT], mybir.dt.bfloat16, tag="CbT")
            xdblT = Sp.tile([96, SCT], F32R, tag="xdblT")

            # -- pass 1: projections + Silu (act set 18) + PSUM copies --
            for (o, w) in CH:
                pxt = psA.tile([128, 512], F32, tag="pbc")
                for j in range(DC):
                    mm(pxt[:, :w], ct["wctap"][r0:r0 + 64, (4 * i + j) * DI:(4 * i + j + 1) * DI],
                       xnh[r0:r0 + 64, o + j:o + j + w],
                       start=(j == 0), stop=(j == DC - 1))
                nc.scalar.activation(xcT[:, o:o + w], pxt[:, :w], AFT.Silu,
                                     bias=ct["ccv"][:, i:i + 1])
                pz = psM.tile([128, 512], F32, tag="pmm")
                mm(pz[:, :w], ct["wz"][r0:r0 + 64, i * DI:(i + 1) * DI],
                   xnh[r0:r0 + 64, o + 3:o + 3 + w])
                nc.scalar.activation(szT[:, o:o + w], pz[:, :w], AFT.Silu,
                                     bias=ct["cz"][:, i:i + 1])
                pxd = psA.tile([96, 512], F32, tag="pbc")
                mm(pxd[:, :w], ct["xpw"][:], xcT[:, o:o + w])
                nc.scalar.copy(xdblT[:, o:o + w], pxd[:, :w])
                pbb = psA.tile([128, 512], F32, tag="pbc")
                mm(pbb[:, :w], ct["patsbc"][32:48, 0:128], xdblT[32:48, o:o + w])
                nc.scalar.copy(BbT[:, o:o + w], pbb[:, :w])
                pcb = psM.tile([128, 512], F32, tag="pmm")
                mm(pcb[:, :w], ct["patsbc"][64:80, 128:256], xdblT[64:80, o:o + w])
                nc.scalar.copy(CbT[:, o:o + w], pcb[:, :w])

            # -- pass 2: dt softplus (act set 6: Exp/Ln) + dtu on Pool --
            for (o, w) in CH:
                pdt = psM.tile([128, 512], F32, tag="pmm")
                mm(pdt[:, :w], ct["dtw"][:], xdblT[0:4, o:o + w])
                # softplus(x) = x + ln(1 + exp(-x)); x = pdt + dtb
                eneg = Tp.tile([128, 512], F32, tag="spe", bufs=1)
                nc.scalar.activation(eneg[:, :w], pdt[:, :w], AFT.Exp, scale=-1.0,
                                     bias=ct["ndtb"][:])
                lnv = Tp.tile([128, 512], F32, tag="spl", bufs=1)
                nc.scalar.activation(lnv[:, :w], eneg[:, :w], AFT.Ln, bias=1.0)
                nc.vector.scalar_tensor_tensor(dtT[:, o:o + w], pdt[:, :w],
                                               ct["dtb"][:], lnv[:, :w],
                                               AOT.add, AOT.add)
                nc.gpsimd.tensor_tensor(dtuT[:, o:o + w], dtT[:, o:o + w].bitcast(F32),
                                        xcT[:, o:o + w].bitcast(F32), AOT.mult)

            # ---- selective scan over 16 (d-group) x 16 (state) partitions ----
            if STAGE <= 2:
                if i == 0:
                    nc.sync.dma_start(out=out[0:128, :], in_=dtT[:, 64:].bitcast(F32))
                    nc.sync.dma_start(out=out[128:256, :], in_=BbT[:, 64:])
                continue
            pY = psY.tile([128, TH], F32, tag="py")
            for g in range(16):
                hT = Cp.tile([128, SCT], F32, tag="hT", bufs=2)
                for (o, w) in CH:
                    aT = Cp.tile([128, 512], F32, tag="aT")
                    bT = Cp.tile([128, 512], F32, tag="bT")
                    pda = psA.tile([128, 512], F32, tag="pbc")
                    mm(pda[:, :w], ct["patg"][:, 128 * g:128 * (g + 1)], dtT[:, o:o + w])
                    nc.scalar.activation(aT[:, :w], pda[:, :w], AFT.Exp,
                                         scale=ct["acols"][:, g:g + 1])
                    pdu = psM.tile([128, 512], F32, tag="pmm")
                    mm(pdu[:, :w], ct["patg"][:, 128 * g:128 * (g + 1)], dtuT[:, o:o + w])
                    nc.vector.scalar_tensor_tensor(bT[:, :w], pdu[:, :w], 1.0,
                                                   BbT[:, o:o + w],
                                                   AOT.mult, AOT.mult)
                    ini = 0.0 if o == 0 else hT[:, o - 1:o]
                    nc.vector.tensor_tensor_scan(hT[:, o:o + w], aT[:, :w], bT[:, :w],
                                                 ini, AOT.mult, AOT.add)
                for c in range(4):
                    o = OSUB * c
                    hcT = Tp.tile([128, OSUB], F32R, tag="hcR")
                    nc.gpsimd.tensor_tensor(hcT[:], hT[:, 64 + o:64 + o + OSUB],
                                            CbT[:, 64 + o:64 + o + OSUB], AOT.mult)
                    mm(pY[:, o:o + OSUB], ct["patyg"][:, 128 * g:128 * (g + 1)],
                       hcT[:], start=(g == 0), stop=(g == 15))

            if STAGE <= 3:
                if i == 0:
                    ySB = Sp.tile([128, TH], F32, tag="oSB")
                    for c in range(4):
                        o = OSUB * c
                        nc.scalar.copy(ySB[:, o:o + OSUB], pY[:, o:o + OSUB])
                    nc.sync.dma_start(out=out[0:128, :], in_=ySB[:])
                    nc.sync.dma_start(out=out[128:256, :], in_=CbT[:, 64:])
                continue
            # ---- gating, out_proj, LN1 stats (act set 3: Square/Sqrt) ----
            mf_t = mfin[i // 2]
            t5 = Tp.tile([128, TH], F32, tag="t5c", bufs=1)
            nc.vector.scalar_tensor_tensor(t5[:], xcT[:, 64:64 + TH].bitcast(F32),
                                           ct["dp"][:], pY[:, 0:TH],
                                           AOT.mult, AOT.add)
            t6 = Tp.tile([128, TH], F32R, tag="t6c", bufs=1)
            nc.gpsimd.tensor_tensor(t6[:], t5[:], szT[:, 64:64 + TH], AOT.mult)
            mnT = Sp.tile([64, TH], F32R, tag="mnT")
            for c in range(4):
                o = OSUB * c
                pm = psM.tile([64, OSUB], F32, tag="pmm")
                mm(pm[:], ct["opw"][:], t6[:, o:o + OSUB])
                mSB = Tp.tile([64, OSUB], F32R, tag="mSBc")
                nc.scalar.copy(mSB[:], pm[:])
                ps1 = psM.tile([1, OSUB], F32, tag="pmm")
                mm(ps1[:], ct["onesc"][0:64, :], mSB[:])
                s1 = Tp.tile([1, 512], F32, tag="rA", bufs=1)
                nc.vector.tensor_scalar(out=s1[:], in0=ps1[:],
                                        scalar1=1.0 / D, scalar2=None, op0=AOT.mult)
                sqt = Tp.tile([64, OSUB], F32R, tag="scrR", bufs=1)
                nc.scalar.activation(sqt[:], mSB[:].bitcast(F32), AFT.Square)
                pq1 = psM.tile([1, OSUB], F32, tag="pmm")
                mm(pq1[:], ct["onesc"][0:64, :], sqt[:])
                q1 = Tp.tile([1, 512], F32, tag="rB", bufs=1)
                nc.vector.tensor_scalar(out=q1[:], in0=pq1[:],
                                        scalar1=1.0 / D, scalar2=None, op0=AOT.mult)
                m2b = Tp.tile([1, 512], F32, tag="rC", bufs=1)
                nc.vector.tensor_tensor(m2b[:], s1[:], s1[:], AOT.mult)
                v1 = Tp.tile([1, 512], F32, tag="rD", bufs=1)
                nc.vector.tensor_tensor(v1[:], q1[:], m2b[:], AOT.subtract)
                sd1 = Tp.tile([1, 512], F32, tag="rC", bufs=1)
                nc.scalar.activation(sd1[:], v1[:], AFT.Sqrt, bias=eps_t[:])
                rs1 = Tp.tile([1, 512], F32, tag="rD", bufs=1)
                nc.vector.reciprocal_approx_fast(rs1[:], sd1[:])
                bmn = Tp.tile([64, OSUB], F32, tag="bcA", bufs=1)
                nc.gpsimd.partition_broadcast(bmn[:], s1[:])
                brs = Tp.tile([64, OSUB], F32, tag="bcB", bufs=1)
                nc.gpsimd.partition_broadcast(brs[:], rs1[:])
                tq = Tp.tile([64, OSUB], F32, tag="scr")
                nc.vector.scalar_tensor_tensor(tq[:], mSB[:].bitcast(F32), 1.0,
                                               bmn[:], AOT.mult, AOT.subtract)
                nc.vector.scalar_tensor_tensor(mnT[:, o:o + OSUB], tq[:], 1.0,
                                               brs[:], AOT.mult, AOT.mult)
            # ---- MLP (act set 10: Gelu) + skip ----
            for c in range(4):
                o = OSUB * c
                ph1 = psM.tile([128, OSUB], F32, tag="pmm")
                mm(ph1[:], ct["fc1"][:, 0:128], mnT[:, o:o + OSUB])
                h1 = Tp.tile([128, OSUB], F32R, tag="h1a")
                nc.scalar.activation(h1[:], ph1[:], AFT.Gelu, bias=ct["fc1b"][:, 0:1])
                ph2 = psM.tile([128, OSUB], F32, tag="pmm")
                mm(ph2[:], ct["fc1"][:, 128:256], mnT[:, o:o + OSUB])
                h2 = Tp.tile([128, OSUB], F32R, tag="h1b")
                nc.scalar.activation(h2[:], ph2[:], AFT.Gelu, bias=ct["fc1b"][:, 1:2])
                pf2 = psM.tile([64, OSUB], F32, tag="pmm")
                mm(pf2[:], ct["fc2"][:, 0:64], h1[:],
                   start=True, stop=False)
                mm(pf2[:], ct["fc2"][:, 64:128], h2[:],
                   start=False, stop=False)
                mm(pf2[:], ct["fc2br"][:], ct["onesr"][:],
                   start=False, stop=True)
                nc.vector.scalar_tensor_tensor(mf_t[r0:r0 + 64, o:o + OSUB],
                                               xnh[r0:r0 + 64, PAD + o:PAD + o + OSUB].bitcast(F32),
                                               ct["skips"][r0:r0 + 64, :],
                                               pf2[:], AOT.mult, AOT.add)

        if STAGE == 4:
            for half in range(2):
                nc.sync.dma_start(out=out[128 * half:128 * (half + 1), :],
                                  in_=mfin[half][:].bitcast(F32))
        # ==== 1x1 conv across chunks + BN + SiLU ====
        for half in range(2 if STAGE >= 5 else 0):
            oSB = Sp.tile([128, TH], F32, tag="oSB")
            for c in range(4):
                o = OSUB * c
                pyc = psM.tile([128, OSUB], F32, tag="pmm")
                for t in range(2):
                    mm(pyc[:], ct["wout"][:, t * C + 128 * half:t * C + 128 * (half + 1)],
                       mfin[t][:, o:o + OSUB], start=(t == 0), stop=(t == 1))
                nc.scalar.activation(oSB[:, o:o + OSUB], pyc[:], AFT.Silu,
                                     scale=ct["bnsc"][:, half:half + 1],
                                     bias=ct["bnsh"][:, half:half + 1])
            nc.sync.dma_start(out=out[128 * half:128 * (half + 1), :], in_=oSB[:])

    nc.compile()
    _cache["nc"] = nc
    return nc


def _host_prep(inputs):
    f32 = np.float32

    def a(k):
        return np.asarray(inputs[k], f32)

    g, b_, Win = a("ln_g"), a("ln_b"), a("in_proj_w")
    convw, convb = a("conv_w"), a("conv_b")
    com = {}
    wctap = np.zeros((D, 16 * DI), f32)
    wz = np.zeros((D, 4 * DI), f32)
    ccv = np.zeros((DI, 4), f32)
    cz = np.zeros((DI, 4), f32)
    for i in range(4):
        gi, bi = g[64 * i:64 * (i + 1)], b_[64 * i:64 * (i + 1)]
        wxc = gi[:, None] * Win[:, :DI]
        for j in range(DC):
            wctap[:, (4 * i + j) * DI:(4 * i + j + 1) * DI] = wxc * convw[None, :, j]
        wz[:, i * DI:(i + 1) * DI] = gi[:, None] * Win[:, DI:]
        ccv[:, i] = (bi @ Win[:, :DI]) * convw.sum(1) + convb
        cz[:, i] = bi @ Win[:, DI:]
    com["wctap"], com["wz"] = np.tile(wctap, (2, 1)), np.tile(wz, (2, 1))
    com["ccv"], com["cz"] = ccv, cz
    xpw_raw = a("x_proj_w")
    xpw = np.zeros((DI, 96), f32)
    xpw[:, 0:DTR] = xpw_raw[:, 0:DTR]
    xpw[:, 32:48] = xpw_raw[:, DTR:DTR + DS]
    xpw[:, 64:80] = xpw_raw[:, DTR + DS:]
    com["xpw"] = xpw
    com["dtw"] = a("dt_proj_w")
    com["dtb"] = a("dt_proj_b").reshape(DI, 1)
    com["ndtb"] = -a("dt_proj_b").reshape(DI, 1)
    A = -np.exp(a("A_log"))
    acols = np.zeros((128, 16), f32)
    for p in range(128):
        for gg in range(16):
            acols[p, gg] = A[8 * gg + p // 16, p % 16]
    com["acols"] = acols
    com["dp"] = a("Dparam").reshape(DI, 1)
    com["opw"] = a("out_proj_w")
    g1, b1, fc1w = a("ln1_g"), a("ln1_b"), a("fc1_w")
    com["fc1"] = g1[:, None] * fc1w
    com["fc1b"] = (a("fc1_b") + b1 @ fc1w).reshape(2, 128).T.copy()
    fc2w = a("fc2_w")
    com["fc2"] = np.concatenate([fc2w[0:128, :], fc2w[128:256, :]], axis=1)
    com["fc2br"] = a("fc2_b").reshape(1, D)
    outcw = a("outc_w")
    wout = np.zeros((128, 2 * C), f32)
    for t in range(2):
        for i in (2 * t, 2 * t + 1):
            for d in range(D):
                wout[64 * (i % 2) + d, t * C:(t + 1) * C] = outcw[:, 4 * d + i]
    com["wout"] = wout
    sc = a("bn_g") / np.sqrt(a("bn_v") + EPS)
    com["bnsc"] = sc.reshape(2, 128).T.copy()
    com["bnsh"] = (a("bn_b") - a("bn_m") * sc).reshape(2, 128).T.copy()
    patg = np.zeros((128, 16 * 128), f32)
    patyg = np.zeros((128, 16 * 128), f32)
    for gg in range(16):
        for p in range(128):
            patg[8 * gg + p // 16, 128 * gg + p] = 1.0    # bcast d-row -> (d,s)
            patyg[p, 128 * gg + 8 * gg + p // 16] = 1.0   # sum over s -> d row
    patsbc = np.zeros((128, 256), f32)
    for p in range(128):
        patsbc[32 + p % 16, p] = 1.0          # B bcast lhsT rows 32:48
        patsbc[64 + p % 16, 128 + p] = 1.0    # C bcast lhsT rows 64:80
    com["patg"], com["patyg"], com["patsbc"] = patg, patyg, patsbc
    com["onesr"] = np.ones((1, 512), f32)
    com["onesc"] = np.ones((128, 1), f32)
    com["skips"] = np.full((128, 1), float(np.asarray(inputs["skip_scale"]).reshape(-1)[0]), f32)
    return {k: np.ascontiguousarray(v, f32) for k, v in com.items()}


def kernel(**inputs):
    nc = _build()
    com = _host_prep(inputs)
    x = np.asarray(inputs["x"], np.float32).reshape(B, C, N)
    in_maps = []
    for k in range(8):
        b, half = k // 2, k % 2
        if half == 0:
            xs = np.concatenate([np.zeros((C, PAD), np.float32), x[b, :, :TH]], axis=1)
        else:
            xs = x[b, :, TH - PAD:N]
        m = {"xs": np.ascontiguousarray(xs)}
        m.update(com)
        in_maps.append(m)
    res = run_bass_kernel_spmd(nc, in_maps, core_ids=list(range(8)))
    outp = np.zeros((B, C, N), np.float32)
    for k in range(8):
        b, half = k // 2, k % 2
        outp[b, :, half * TH:(half + 1) * TH] = res.results[k]["out"]
    return outp.reshape(B, C, H, W)


# revision 18
# speedup vs baseline: 1.7892x; 1.0263x over previous
"""Trainium2 Bass kernel for nn_CSI_75453985457421 (LN + chunked Mamba + MLP + 1x1conv + BN + SiLU).

Sharding: 8 cores = (batch b 0..3) x (time-half 0..1). Each core gets
x[b, :, half*2048-67 : half*2048+2048] (zero-padded before the sequence start)
and computes its 2048 output positions independently: 67 warmup columns
(3 causal-conv pad + 64 scan warmup; state decay <= exp(-0.68*64) << fp32 eps).

Device layout: time on the free axis. The selective scan runs with partitions
= (d_local, s): 16 groups of 8 d-channels x 16 states via the hardware
tensor_tensor_scan (DVE). dt/dtu/B/C broadcasts and the final sum over s are
TensorE pattern matmuls in float32r (1 cycle/row vs 4 for fp32); exp(A*dt) is
ScalarE with a per-partition scale. LN gamma/beta, the depthwise conv, the
channel interleave and BatchNorm are folded into weights on the host.
Stat broadcasts run on the idle GpSimd/Pool engine (partition_broadcast).
"""
import os
import sys

sys.path.insert(0, "/opt/trn_rl_repo")
STAGE = int(os.environ.get("KSTAGE", "9"))
import numpy as np
import concourse.bass as bass
import concourse.bacc as bacc
import concourse.tile as tile
from concourse import mybir
from concourse.bass_utils import run_bass_kernel_spmd

F32 = mybir.dt.float32
F32R = mybir.dt.float32r
AOT = mybir.AluOpType
AFT = mybir.ActivationFunctionType

B, C, H, W = 4, 256, 64, 64
N = H * W
D, DI, DS, DC, DTR, MH = 64, 128, 16, 4, 4, 256
EPS = 1e-5
PAD = 67
TH = 2048
TEXT = PAD + TH          # 2115
SCT = TEXT - 3           # 2112 = 4*528
SUB = 528
OSUB = 512

import functools
import concourse.hw_specs as _hw_specs
import concourse.bacc as _bacc_mod

_real_get_tables = _hw_specs.get_activation_tables

@functools.cache
def _patched_get_tables(arch):
    out = {}
    for name, s in _real_get_tables(arch).items():
        s = set(s)
        if name in ("exp_and_others", "exp_and_friends"):
            s.discard(AFT.Exp)
        if name == "natural_log":
            s.discard(AFT.Ln)
        out[name] = s
    return out

_hw_specs.get_activation_tables = _patched_get_tables
_bacc_mod.get_activation_tables = _patched_get_tables

_cache = {}

# name -> (shape, is_matmul_operand)
_IN_SHAPES = dict(
    xs=((C, TEXT), True), wctap=((128, 16 * DI), True), wz=((128, 4 * DI), True),
    ccv=((DI, 4), False), cz=((DI, 4), False),
    xpw=((DI, 96), True), dtw=((DTR, DI), True), dtb=((DI, 1), False),
    acols=((128, 16), False), dp=((DI, 1), False), ndtb=((DI, 1), False),
    opw=((DI, D), True), fc1=((D, MH), True), fc1b=((128, 2), False),
    fc2=((128, 2 * D), True), fc2br=((1, D), True),
    wout=((128, 2 * C), True), bnsc=((128, 2), False), bnsh=((128, 2), False),
    patg=((128, 16 * 128), "bf"), patyg=((128, 16 * 128), "bf"),
    patsbc=((128, 256), True), onesr=((1, 512), True), onesc=((128, 1), True),
    skips=((128, 1), False),
)


def _build():
    if "nc" in _cache:
        return _cache["nc"]
    nc = bacc.Bacc("TRN2", target_bir_lowering=False, debug=False, num_devices=8)
    BF16 = mybir.dt.bfloat16
    U16 = mybir.dt.uint16
    dram = {k: nc.dram_tensor(k, list(s), U16 if r == "bf" else F32,
                              kind="ExternalInput").ap()
            for k, (s, r) in _IN_SHAPES.items()}
    out = nc.dram_tensor("out", [C, TH], F32, kind="ExternalOutput").ap()

    with tile.TileContext(nc) as tc, \
            tc.tile_pool(name="const", bufs=1) as Kp, \
            tc.tile_pool(name="big", bufs=1) as Bp, \
            tc.tile_pool(name="seq", bufs=1) as Sp, \
            tc.tile_pool(name="tmp", bufs=2) as Tp, \
            tc.tile_pool(name="scan", bufs=2) as Cp, \
            tc.tile_pool(name="psA", bufs=2, space="PSUM") as psA, \
            tc.tile_pool(name="psM", bufs=2, space="PSUM") as psM, \
            tc.tile_pool(name="psY", bufs=1, space="PSUM") as psY:

        def mm(out_ap, lhsT, rhs, start=True, stop=True):
            n = out_ap.shape[-1]
            if n <= 512:
                nc.tensor.matmul(out_ap, lhsT, rhs, start=start, stop=stop)
                return
            o = 0
            while o < n:
                w_ = min(512, n - o)
                nc.tensor.matmul(out_ap[..., o:o + w_], lhsT, rhs[..., o:o + w_],
                                 start=start, stop=stop)
                o += w_

        ct = {}
        for k, (shp, is_r) in _IN_SHAPES.items():
            if k == "xs":
                continue
            dt_ = {True: F32R, False: F32, "bf": BF16}[is_r]
            ct[k] = Kp.tile(list(shp), dt_, tag=k, name=f"ct_{k}")
            srcap = dram[k][:] if is_r is False else dram[k][:].bitcast(dt_)
            nc.sync.dma_start(out=ct[k][:], in_=srcap)
        eps_t = Kp.tile([1, 1], F32, tag="eps")
        nc.vector.memset(eps_t[:], EPS)

        xh = [Bp.tile([128, TEXT], F32R, tag=f"xh{h}", name=f"xh{h}") for h in range(2)]
        for h in range(2):
            nc.sync.dma_start(out=xh[h][:], in_=dram["xs"][128 * h:128 * (h + 1), :].bitcast(F32R))

        # ---- LayerNorm over C: fused per-subtile stats + apply ----
        # last subtile overlaps col 2047 so every width stays even (fp32r
        # matmul requires an even moving width); re-normalizing an already
        # normalized column is a ~eps no-op.
        nsub = [(0, 512), (512, 512), (1024, 512), (1536, 512), (TEXT - 68, 68)]
        for (o, w_) in nsub:
            pse = psM.tile([1, 512], F32, tag="pmm")
            for h in range(2):
                mm(pse[:, :w_], ct["onesc"][:], xh[h][:, o:o + w_],
                   start=(h == 0), stop=(h == 1))
            mean = Tp.tile([1, 512], F32, tag="rA", bufs=1)
            nc.vector.tensor_scalar(out=mean[:, :w_], in0=pse[:, :w_],
                                    scalar1=1.0 / C, scalar2=None, op0=AOT.mult)
            psq = psM.tile([1, 512], F32, tag="pmm")
            for h in range(2):
                sqt = Tp.tile([128, 512], F32R, tag="scrR", bufs=1)
                nc.scalar.activation(sqt[:, :w_], xh[h][:, o:o + w_].bitcast(F32),
                                     AFT.Square)
                mm(psq[:, :w_], ct["onesc"][:], sqt[:, :w_],
                   start=(h == 0), stop=(h == 1))
            sqm = Tp.tile([1, 512], F32, tag="rB", bufs=1)
            nc.vector.tensor_scalar(out=sqm[:, :w_], in0=psq[:, :w_],
                                    scalar1=1.0 / C, scalar2=None, op0=AOT.mult)
            m2 = Tp.tile([1, 512], F32, tag="rC", bufs=1)
            nc.vector.tensor_tensor(m2[:, :w_], mean[:, :w_], mean[:, :w_], AOT.mult)
            var = Tp.tile([1, 512], F32, tag="rD", bufs=1)
            nc.vector.tensor_tensor(var[:, :w_], sqm[:, :w_], m2[:, :w_], AOT.subtract)
            sd = Tp.tile([1, 512], F32, tag="rC", bufs=1)
            nc.scalar.activation(sd[:, :w_], var[:, :w_], AFT.Sqrt, bias=eps_t[:])
            rstd = Tp.tile([1, 512], F32, tag="rD", bufs=1)
            nc.vector.reciprocal_approx_fast(rstd[:, :w_], sd[:, :w_])
            bmean = Tp.tile([128, 512], F32, tag="bcA", bufs=1)
            nc.gpsimd.partition_broadcast(bmean[:, :w_], mean[:, :w_])
            brstd = Tp.tile([128, 512], F32, tag="bcB", bufs=1)
            nc.gpsimd.partition_broadcast(brstd[:, :w_], rstd[:, :w_])
            for h in range(2):
                tmp = Tp.tile([128, 512], F32, tag="scr")
                nc.vector.scalar_tensor_tensor(tmp[:, :w_], xh[h][:, o:o + w_].bitcast(F32),
                                               1.0, bmean[:, :w_], AOT.mult, AOT.subtract)
                nc.vector.scalar_tensor_tensor(xh[h][:, o:o + w_], tmp[:, :w_], 1.0,
                                               brstd[:, :w_], AOT.mult, AOT.mult)

        mfin = [Bp.tile([128, TH], F32R, tag=f"mfin{h}", name=f"mfin{h}") for h in range(2)]
        if STAGE <= 1:
            for half in range(2):
                nc.sync.dma_start(out=out[128 * half:128 * (half + 1), :],
                                  in_=xh[half][:, PAD:].bitcast(F32))
        CH = [(0, 64), (64, 512), (576, 512), (1088, 512), (1600, 512)]
        nseq = 0 if STAGE <= 1 else 4
        # ==== per sequence (channel chunk) ====
        # Emission is batched by activation-table set to minimize 1.3us
        # LoadActFuncSet switches: pass1 Silu/Copy (set 18), pass2+scan
        # Exp/Ln (set 6), MLP stats Square/Sqrt (set 3), Gelu (set 10).
        for i in range(nseq):
            xnh = xh[i // 2]
            r0 = 64 * (i % 2)
            xcT = Sp.tile([128, SCT], F32R, tag="xcT", bufs=2)
            szT = Sp.tile([128, SCT], mybir.dt.bfloat16, tag="szT", bufs=2)
            dtT = Sp.tile([128, SCT], mybir.dt.bfloat16, tag="dtT")
            dtuT = Sp.tile([128, SCT], mybir.dt.bfloat16, tag="dtuT")
            BbT = Sp.tile([128, SCT], mybir.dt.bfloat16, tag="BbT", bufs=2)
            CbT = Sp.tile([128, SCT], mybir.dt.bfloat16, tag="CbT", bufs=2)
            xdblT = Sp.tile([96, SCT], F32R, tag="xdblT", bufs=2)

            # -- pass 1: projections + Silu (act set 18) + PSUM copies --
            for (o, w) in CH:
                pxt = psA.tile([128, 512], F32, tag="pbc")
                for j in range(DC):
                    mm(pxt[:, :w], ct["wctap"][r0:r0 + 64, (4 * i + j) * DI:(4 * i + j + 1) * DI],
                       xnh[r0:r0 + 64, o + j:o + j + w],
                       start=(j == 0), stop=(j == DC - 1))
                nc.scalar.activation(xcT[:, o:o + w], pxt[:, :w], AFT.Silu,
                                     bias=ct["ccv"][:, i:i + 1])
                pz = psM.tile([128, 512], F32, tag="pmm")
                mm(pz[:, :w], ct["wz"][r0:r0 + 64, i * DI:(i + 1) * DI],
                   xnh[r0:r0 + 64, o + 3:o + 3 + w])
                nc.scalar.activation(szT[:, o:o + w], pz[:, :w], AFT.Silu,
                                     bias=ct["cz"][:, i:i + 1])
                pxd = psA.tile([96, 512], F32, tag="pbc")
                mm(pxd[:, :w], ct["xpw"][:], xcT[:, o:o + w])
                nc.scalar.copy(xdblT[:, o:o + w], pxd[:, :w])
                pbb = psA.tile([128, 512], F32, tag="pbc")
                mm(pbb[:, :w], ct["patsbc"][32:48, 0:128], xdblT[32:48, o:o + w])
                nc.scalar.copy(BbT[:, o:o + w], pbb[:, :w])
                pcb = psM.tile([128, 512], F32, tag="pmm")
                mm(pcb[:, :w], ct["patsbc"][64:80, 128:256], xdblT[64:80, o:o + w])
                nc.scalar.copy(CbT[:, o:o + w], pcb[:, :w])

            # -- pass 2: dt softplus (act set 6: Exp/Ln) + dtu on Pool --
            for (o, w) in CH:
                pdt = psM.tile([128, 512], F32, tag="pmm")
                mm(pdt[:, :w], ct["dtw"][:], xdblT[0:4, o:o + w])
                # softplus(x) = x + ln(1 + exp(-x)); x = pdt + dtb
                eneg = Tp.tile([128, 512], F32, tag="spe", bufs=1)
                nc.scalar.activation(eneg[:, :w], pdt[:, :w], AFT.Exp, scale=-1.0,
                                     bias=ct["ndtb"][:])
                lnv = Tp.tile([128, 512], F32, tag="spl", bufs=1)
                nc.scalar.activation(lnv[:, :w], eneg[:, :w], AFT.Ln, bias=1.0)
                nc.vector.scalar_tensor_tensor(dtT[:, o:o + w], pdt[:, :w],
                                               ct["dtb"][:], lnv[:, :w],
                                               AOT.add, AOT.add)
                nc.gpsimd.tensor_tensor(dtuT[:, o:o + w], dtT[:, o:o + w],
                                        xcT[:, o:o + w].bitcast(F32), AOT.mult)

            # ---- selective scan over 16 (d-group) x 16 (state) partitions ----
            if STAGE <= 2:
                if i == 0:
                    nc.sync.dma_start(out=out[0:128, :], in_=xcT[:, 64:].bitcast(F32))
                    nc.sync.dma_start(out=out[128:256, :], in_=xcT[:, 64:].bitcast(F32))
                continue
            pY = psY.tile([128, TH], F32, tag="py")
            for g in range(16):
                hT = Cp.tile([128, SCT], mybir.dt.bfloat16, tag="hT", bufs=2)
                for (o, w) in CH:
                    aT = Cp.tile([128, 512], F32, tag="aT")
                    bT = Cp.tile([128, 512], F32, tag="bT")
                    pda = psA.tile([128, 512], F32, tag="pbc")
                    mm(pda[:, :w], ct["patg"][:, 128 * g:128 * (g + 1)], dtT[:, o:o + w])
                    nc.scalar.activation(aT[:, :w], pda[:, :w], AFT.Exp,
                                         scale=ct["acols"][:, g:g + 1])
                    pdu = psM.tile([128, 512], F32, tag="pmm")
                    mm(pdu[:, :w], ct["patg"][:, 128 * g:128 * (g + 1)], dtuT[:, o:o + w])
                    nc.vector.scalar_tensor_tensor(bT[:, :w], pdu[:, :w], 1.0,
                                                   BbT[:, o:o + w],
                                                   AOT.mult, AOT.mult)
                    ini = 0.0 if o == 0 else hT[:, o - 1:o]
                    nc.vector.tensor_tensor_scan(hT[:, o:o + w], aT[:, :w], bT[:, :w],
                                                 ini, AOT.mult, AOT.add)
                for c in range(4):
                    o = OSUB * c
                    hcT = Tp.tile([128, OSUB], mybir.dt.bfloat16, tag="hcR")
                    nc.gpsimd.tensor_tensor(hcT[:], hT[:, 64 + o:64 + o + OSUB],
                                            CbT[:, 64 + o:64 + o + OSUB], AOT.mult)
                    mm(pY[:, o:o + OSUB], ct["patyg"][:, 128 * g:128 * (g + 1)],
                       hcT[:], start=(g == 0), stop=(g == 15))

            if STAGE <= 3:
                if i == 0:
                    ySB = Tp.tile([128, TH], F32, tag="t5c", bufs=1)
                    for c in range(4):
                        o = OSUB * c
                        nc.scalar.copy(ySB[:, o:o + OSUB], pY[:, o:o + OSUB])
                    nc.sync.dma_start(out=out[0:128, :], in_=ySB[:])
                    nc.sync.dma_start(out=out[128:256, :], in_=ySB[:])
                continue
            # ---- gating, out_proj, LN1 stats (act set 3: Square/Sqrt) ----
            mf_t = mfin[i // 2]
            t5 = Tp.tile([128, TH], F32, tag="t5c", bufs=1)
            nc.vector.scalar_tensor_tensor(t5[:], xcT[:, 64:64 + TH].bitcast(F32),
                                           ct["dp"][:], pY[:, 0:TH],
                                           AOT.mult, AOT.add)
            t6 = Tp.tile([128, TH], F32R, tag="t6c", bufs=1)
            nc.gpsimd.tensor_tensor(t6[:], t5[:], szT[:, 64:64 + TH], AOT.mult)
            mnT = Sp.tile([64, TH], F32R, tag="mnT")
            for c in range(4):
                o = OSUB * c
                pm = psM.tile([64, OSUB], F32, tag="pmm")
                mm(pm[:], ct["opw"][:], t6[:, o:o + OSUB])
                mSB = Tp.tile([64, OSUB], F32R, tag="mSBc")
                nc.scalar.copy(mSB[:], pm[:])
                ps1 = psM.tile([1, OSUB], F32, tag="pmm")
                mm(ps1[:], ct["onesc"][0:64, :], mSB[:])
                s1 = Tp.tile([1, 512], F32, tag="rA", bufs=1)
                nc.vector.tensor_scalar(out=s1[:], in0=ps1[:],
                                        scalar1=1.0 / D, scalar2=None, op0=AOT.mult)
                sqt = Tp.tile([64, OSUB], F32R, tag="scrR", bufs=1)
                nc.scalar.activation(sqt[:], mSB[:].bitcast(F32), AFT.Square)
                pq1 = psM.tile([1, OSUB], F32, tag="pmm")
                mm(pq1[:], ct["onesc"][0:64, :], sqt[:])
                q1 = Tp.tile([1, 512], F32, tag="rB", bufs=1)
                nc.vector.tensor_scalar(out=q1[:], in0=pq1[:],
                                        scalar1=1.0 / D, scalar2=None, op0=AOT.mult)
                m2b = Tp.tile([1, 512], F32, tag="rC", bufs=1)
                nc.vector.tensor_tensor(m2b[:], s1[:], s1[:], AOT.mult)
                v1 = Tp.tile([1, 512], F32, tag="rD", bufs=1)
                nc.vector.tensor_tensor(v1[:], q1[:], m2b[:], AOT.subtract)
                sd1 = Tp.tile([1, 512], F32, tag="rC", bufs=1)
                nc.scalar.activation(sd1[:], v1[:], AFT.Sqrt, bias=eps_t[:])
                rs1 = Tp.tile([1, 512], F32, tag="rD", bufs=1)
                nc.vector.reciprocal_approx_fast(rs1[:], sd1[:])
                bmn = Tp.tile([64, OSUB], F32, tag="bcA", bufs=1)
                nc.gpsimd.partition_broadcast(bmn[:], s1[:])
                brs = Tp.tile([64, OSUB], F32, tag="bcB", bufs=1)
                nc.gpsimd.partition_broadcast(brs[:], rs1[:])
                tq = Tp.tile([64, OSUB], F32, tag="scr")
                nc.vector.scalar_tensor_tensor(tq[:], mSB[:].bitcast(F32), 1.0,
                                               bmn[:], AOT.mult, AOT.subtract)
                nc.vector.scalar_tensor_tensor(mnT[:, o:o + OSUB], tq[:], 1.0,
                                               brs[:], AOT.mult, AOT.mult)
            # ---- MLP (act set 10: Gelu) + skip ----
            for c in range(4):
                o = OSUB * c
                ph1 = psM.tile([128, OSUB], F32, tag="pmm")
                mm(ph1[:], ct["fc1"][:, 0:128], mnT[:, o:o + OSUB])
                h1 = Tp.tile([128, OSUB], F32R, tag="h1a")
                nc.scalar.activation(h1[:], ph1[:], AFT.Gelu, bias=ct["fc1b"][:, 0:1])
                ph2 = psM.tile([128, OSUB], F32, tag="pmm")
                mm(ph2[:], ct["fc1"][:, 128:256], mnT[:, o:o + OSUB])
                h2 = Tp.tile([128, OSUB], F32R, tag="h1b")
                nc.scalar.activation(h2[:], ph2[:], AFT.Gelu, bias=ct["fc1b"][:, 1:2])
                pf2 = psM.tile([64, OSUB], F32, tag="pmm")
                mm(pf2[:], ct["fc2"][:, 0:64], h1[:],
                   start=True, stop=False)
                mm(pf2[:], ct["fc2"][:, 64:128], h2[:],
                   start=False, stop=False)
                mm(pf2[:], ct["fc2br"][:], ct["onesr"][:],
                   start=False, stop=True)
                nc.vector.scalar_tensor_tensor(mf_t[r0:r0 + 64, o:o + OSUB],
                                               xnh[r0:r0 + 64, PAD + o:PAD + o + OSUB].bitcast(F32),
                                               ct["skips"][r0:r0 + 64, :],
                                               pf2[:], AOT.mult, AOT.add)

        if STAGE == 4:
            for half in range(2):
                nc.sync.dma_start(out=out[128 * half:128 * (half + 1), :],
                                  in_=mfin[half][:].bitcast(F32))
        # ==== 1x1 conv across chunks + BN + SiLU ====
        for half in range(2 if STAGE >= 5 else 0):
            oSB = Tp.tile([128, TH], F32, tag="t5c", bufs=1)
            for c in range(4):
                o = OSUB * c
                pyc = psM.tile([128, OSUB], F32, tag="pmm")
                for t in range(2):
                    mm(pyc[:], ct["wout"][:, t * C + 128 * half:t * C + 128 * (half + 1)],
                       mfin[t][:, o:o + OSUB], start=(t == 0), stop=(t == 1))
                nc.scalar.activation(oSB[:, o:o + OSUB], pyc[:], AFT.Silu,
                                     scale=ct["bnsc"][:, half:half + 1],
                                     bias=ct["bnsh"][:, half:half + 1])
            nc.sync.dma_start(out=out[128 * half:128 * (half + 1), :], in_=oSB[:])

    nc.compile()
    _cache["nc"] = nc
    return nc


def _host_prep(inputs):
    f32 = np.float32

    def a(k):
        return np.asarray(inputs[k], f32)

    g, b_, Win = a("ln_g"), a("ln_b"), a("in_proj_w")
    convw, convb = a("conv_w"), a("conv_b")
    com = {}
    wctap = np.zeros((D, 16 * DI), f32)
    wz = np.zeros((D, 4 * DI), f32)
    ccv = np.zeros((DI, 4), f32)
    cz = np.zeros((DI, 4), f32)
    for i in range(4):
        gi, bi = g[64 * i:64 * (i + 1)], b_[64 * i:64 * (i + 1)]
        wxc = gi[:, None] * Win[:, :DI]
        for j in range(DC):
            wctap[:, (4 * i + j) * DI:(4 * i + j + 1) * DI] = wxc * convw[None, :, j]
        wz[:, i * DI:(i + 1) * DI] = gi[:, None] * Win[:, DI:]
        ccv[:, i] = (bi @ Win[:, :DI]) * convw.sum(1) + convb
        cz[:, i] = bi @ Win[:, DI:]
    com["wctap"], com["wz"] = np.tile(wctap, (2, 1)), np.tile(wz, (2, 1))
    com["ccv"], com["cz"] = ccv, cz
    xpw_raw = a("x_proj_w")
    xpw = np.zeros((DI, 96), f32)
    xpw[:, 0:DTR] = xpw_raw[:, 0:DTR]
    xpw[:, 32:48] = xpw_raw[:, DTR:DTR + DS]
    xpw[:, 64:80] = xpw_raw[:, DTR + DS:]
    com["xpw"] = xpw
    com["dtw"] = a("dt_proj_w")
    com["dtb"] = a("dt_proj_b").reshape(DI, 1)
    com["ndtb"] = -a("dt_proj_b").reshape(DI, 1)
    A = -np.exp(a("A_log"))
    acols = np.zeros((128, 16), f32)
    for p in range(128):
        for gg in range(16):
            acols[p, gg] = A[8 * gg + p // 16, p % 16]
    com["acols"] = acols
    com["dp"] = a("Dparam").reshape(DI, 1)
    com["opw"] = a("out_proj_w")
    g1, b1, fc1w = a("ln1_g"), a("ln1_b"), a("fc1_w")
    com["fc1"] = g1[:, None] * fc1w
    com["fc1b"] = (a("fc1_b") + b1 @ fc1w).reshape(2, 128).T.copy()
    fc2w = a("fc2_w")
    com["fc2"] = np.concatenate([fc2w[0:128, :], fc2w[128:256, :]], axis=1)
    com["fc2br"] = a("fc2_b").reshape(1, D)
    outcw = a("outc_w")
    wout = np.zeros((128, 2 * C), f32)
    for t in range(2):
        for i in (2 * t, 2 * t + 1):
            for d in range(D):
                wout[64 * (i % 2) + d, t * C:(t + 1) * C] = outcw[:, 4 * d + i]
    com["wout"] = wout
    sc = a("bn_g") / np.sqrt(a("bn_v") + EPS)
    com["bnsc"] = sc.reshape(2, 128).T.copy()
    com["bnsh"] = (a("bn_b") - a("bn_m") * sc).reshape(2, 128).T.copy()
    patg = np.zeros((128, 16 * 128), f32)
    patyg = np.zeros((128, 16 * 128), f32)
    for gg in range(16):
        for p in range(128):
            patg[8 * gg + p // 16, 128 * gg + p] = 1.0    # bcast d-row -> (d,s)
            patyg[p, 128 * gg + 8 * gg + p // 16] = 1.0   # sum over s -> d row
    patsbc = np.zeros((128, 256), f32)
    for p in range(128):
        patsbc[32 + p % 16, p] = 1.0          # B bcast lhsT rows 32:48
        patsbc[64 + p % 16, 128 + p] = 1.0    # C bcast lhsT rows 64:80
    one_bf = np.uint16(0x3F80)
    com["patg"] = (patg != 0).astype(np.uint16) * one_bf
    com["patyg"] = (patyg != 0).astype(np.uint16) * one_bf
    com["patsbc"] = patsbc
    com["onesr"] = np.ones((1, 512), f32)
    com["onesc"] = np.ones((128, 1), f32)
    com["skips"] = np.full((128, 1), float(np.asarray(inputs["skip_scale"]).reshape(-1)[0]), f32)
    return {k: np.ascontiguousarray(v) if v.dtype == np.uint16
            else np.ascontiguousarray(v, f32) for k, v in com.items()}


def kernel(**inputs):
    nc = _build()
    com = _host_prep(inputs)
    x = np.asarray(inputs["x"], np.float32).reshape(B, C, N)
    in_maps = []
    for k in range(8):
        b, half = k // 2, k % 2
        if half == 0:
            xs = np.concatenate([np.zeros((C, PAD), np.float32), x[b, :, :TH]], axis=1)
        else:
            xs = x[b, :, TH - PAD:N]
        m = {"xs": np.ascontiguousarray(xs)}
        m.update(com)
        in_maps.append(m)
    res = run_bass_kernel_spmd(nc, in_maps, core_ids=list(range(8)))
    outp = np.zeros((B, C, N), np.float32)
    for k in range(8):
        b, half = k // 2, k % 2
        outp[b, :, half * TH:(half + 1) * TH] = res.results[k]["out"]
    return outp.reshape(B, C, H, W)
